# revision 12
# baseline (speedup 1.0000x reference)
"""GAT layer (gnn_message_passing) on 8 trn2 NeuronCores.

Strategy (dst-sharded, no collectives), v2:
- Each core owns a contiguous 1/8 slice of target nodes; host buckets edges by
  dst core. Within a core, owned nodes are sorted by in-degree (descending) and
  grouped into 128-node windows; node -> SBUF partition, its in-edges occupy
  "slot columns" t=0..deg-1 of that partition.
- Per edge slot, a 768B bf16 row [xp[2j] | xp[2j+1] | a_s[2j] | a_s[2j+1] |
  pad] is fetched with SWDGE dma_gather (idx = perm_pos(src)>>1, int16).
  Gather calls are kept small (<=256 descs) and round-robin over 4 queues to
  stay under the SWDGE ring-stall knee.
- Parity (which half of the pair is the real src) and slot validity are folded
  into an additive logit mask L8 (0 or -100): exp(u - 100) == 0, so no select
  masks and no wasted multiplies.
- Attention logits u = a_s(row) + a_t+biases(pass-0 col) + a_e(edge-attr
  grouped reduce) computed in [128, t, 2, 4] bf16; leaky-relu via STT.
- ACT (Scalar) engine computes exp with a pre-EXPANDED output written straight
  into the rhs buffer in an interleaved [head, 32 feat + 1 denom] = 132-col
  layout; DVE then multiplies rhs by xs in place (STT, 4x bf16 mode) and
  pairwise-folds slots (STT adds). Numerator + denominator come out of one
  fold. Residual x @ W_res.T + bias via ones-row-extended matmul (PE, bf16).
"""
import os
import sys
from contextlib import ExitStack

sys.path.insert(0, "/opt/trn_rl_repo")

import numpy as np

N, E = 50000, 1600000
IN_F, EDGE_F, HEADS, OUT_F = 64, 16, 4, 32
NEG_SLOPE = 0.2
NCORES = 8
NODES_PC = N // NCORES            # 6250
NW = (NODES_PC + 127) // 128      # 49 windows/core
WNODES = NW * 128                 # 6272 (last window partially real)
TC_TILES = 48                     # compute-chunk size in 128-slot tiles
GCALL_TILES = 1                   # tiles per dma_gather call (128 descs)
ROWF = 384                        # bf16 row: 256 xp-pair + 8 a_s + 120 pad
# exp(LMASK) must stay nonzero in bf16 so empty segments keep a nonzero
# denominator (no clamp needed): exp(-80) ~ 1.8e-35 > bf16 min normal.
LMASK = -80.0


def _bf16(a):
    import ml_dtypes
    return np.ascontiguousarray(np.asarray(a, np.float32).astype(ml_dtypes.bfloat16))


def _host_preprocess(x, edge_index, edge_attr, W_lin, w_s, b_s, w_t, b_t,
                     W_edge, w_e, b_e, W_res, bias):
    """Pure index/layout work + weight folding. Returns (common, per_core)."""
    src = edge_index[0].astype(np.int64)
    dst = edge_index[1].astype(np.int64)
    deg = np.bincount(dst, minlength=N)

    # ---- weight folding (weights only; standard operator fusion) ----
    wlinT = np.ascontiguousarray(W_lin.T)                      # [64, 128]
    C = (W_edge.reshape(HEADS, OUT_F, EDGE_F) * w_e[None, :, None]).sum(1)  # [4,16]
    crep = np.tile(C.reshape(-1)[None, :], (128, 1)).astype(np.float32)    # [128,64]
    D = (W_lin.reshape(HEADS, OUT_F, IN_F) * w_t[None, :, None]).sum(1).T  # [64,4]
    b_total = float(b_s) + float(b_t) + float(b_e)
    dext = np.vstack([D, np.full((1, HEADS), b_total, np.float32)]).astype(np.float32)
    Dws = (W_lin.reshape(HEADS, OUT_F, IN_F) * w_s[None, :, None]).sum(1).T  # [64,4]
    wlind = np.concatenate([wlinT.astype(np.float32), Dws.astype(np.float32)],
                           axis=1)                                          # [64,132]
    wrese = np.vstack([W_res.T, bias[None, :]]).astype(np.float32)         # [65,128]

    # ---- per-core schedules (common T_w across cores) ----
    cores = []
    for c in range(NCORES):
        lo = c * NODES_PC
        owned = np.arange(lo, lo + NODES_PC)
        dc = deg[owned]
        order = np.argsort(-dc, kind="stable")
        perm_owned = owned[order]
        degs_sorted = dc[order]
        tw = np.maximum(degs_sorted[::128][:NW], 1).astype(np.int64)
        cores.append(dict(perm_owned=perm_owned, tw=tw))

    T_w = np.max(np.stack([cc["tw"] for cc in cores]), axis=0)  # [NW]
    TOFF = np.concatenate([[0], np.cumsum(T_w)])                # slot col offsets
    SUMT = int(TOFF[-1])

    chunks = []           # (w, t0, t1)
    for w in range(NW):
        t = 0
        while t < T_w[w]:
            t1 = min(t + TC_TILES, int(T_w[w]))
            chunks.append((w, t, t1))
            t = t1

    per_core = []
    for c in range(NCORES):
        cc = cores[c]
        perm_owned = cc["perm_owned"]
        rest = np.setdiff1d(np.arange(N), perm_owned, assume_unique=True)
        perm = np.concatenate([perm_owned, rest])
        perm_pos = np.empty(N, np.int64)
        perm_pos[perm] = np.arange(N)

        emask = (dst >= c * NODES_PC) & (dst < (c + 1) * NODES_PC)
        e_ids = np.nonzero(emask)[0]
        d_loc = perm_pos[dst[e_ids]]                 # 0..6249
        eorder = np.argsort(d_loc, kind="stable")
        e_s = e_ids[eorder]
        ds = d_loc[eorder]
        starts = np.searchsorted(ds, np.arange(NODES_PC))
        t_of = np.arange(len(ds)) - starts[ds]
        w_of = ds // 128
        p_of = ds % 128
        col = TOFF[w_of] + t_of

        src_rel = perm_pos[src[e_s]]
        par = (src_rel & 1).astype(np.int64)

        idx_slot = np.zeros((128, SUMT), np.int16)
        idx_slot[p_of, col] = (src_rel >> 1).astype(np.int16)

        # additive logit mask: [128, SUMT, 2(parity half), 4(heads)]
        l8 = np.full((128, SUMT, 2, HEADS), np.float32(LMASK), np.float32)
        l8[p_of, col, par] = 0.0

        ea_slot = np.zeros((128, SUMT, EDGE_F), np.float32)
        ea_slot[p_of, col] = edge_attr[e_s]

        # idx wrapped in 16 partitions (per window), replicated x8
        idx16 = np.zeros((128, SUMT * 8), np.int16)
        for w in range(NW):
            t0, t1 = int(TOFF[w]), int(TOFF[w + 1])
            flat = idx_slot[:, t0:t1].T.reshape(-1)
            wrapped = flat.reshape(-1, 16).T          # [16, T_w*8]
            idx16[:, t0 * 8: t1 * 8] = np.tile(wrapped, (8, 1))

        xT_ext = np.empty((IN_F + 1, N), np.float32)
        xT_ext[:IN_F] = x[perm].T
        xT_ext[IN_F] = 1.0

        per_core.append(dict(
            xT=_bf16(xT_ext),
            idx16=idx16,
            l8=_bf16(l8.reshape(128, SUMT * 8)),
            ea=_bf16(ea_slot.reshape(128, SUMT * EDGE_F)),
            perm_owned=perm_owned,
        ))

    common = dict(T_w=T_w, TOFF=TOFF, SUMT=SUMT, chunks=chunks,
                  wlind=_bf16(wlind), dext=_bf16(dext), crep=_bf16(crep),
                  wrese=_bf16(wrese))
    return common, per_core


def _build_program(common):
    import concourse.bass as bass
    import concourse.tile as tile
    from concourse import bacc, mybir

    f32 = mybir.dt.float32
    bf16 = mybir.dt.bfloat16
    i16 = mybir.dt.int16
    AL = mybir.AluOpType
    SUMT = common["SUMT"]
    T_w, TOFF, chunks = common["T_w"], common["TOFF"], common["chunks"]

    nc = bacc.Bacc("TRN2", target_bir_lowering=False, debug=False,
                   num_devices=NCORES, num_swdge_queues=4)

    xT_d = nc.dram_tensor("xT", [IN_F + 1, N], bf16, kind="ExternalInput")
    idx_d = nc.dram_tensor("idx16", [128, SUMT * 8], i16, kind="ExternalInput")
    l8_d = nc.dram_tensor("l8", [128, SUMT * 8], bf16, kind="ExternalInput")
    ea_d = nc.dram_tensor("ea", [128, SUMT * EDGE_F], bf16, kind="ExternalInput")
    wlin_d = nc.dram_tensor("wlind", [IN_F, 132], bf16, kind="ExternalInput")
    dext_d = nc.dram_tensor("dext", [IN_F + 1, HEADS], bf16, kind="ExternalInput")
    crep_d = nc.dram_tensor("crep", [128, HEADS * EDGE_F], bf16, kind="ExternalInput")
    wrese_d = nc.dram_tensor("wrese", [IN_F + 1, 128], bf16, kind="ExternalInput")
    out_d = nc.dram_tensor("out", [WNODES, 128], f32, kind="ExternalOutput")

    with tile.TileContext(nc) as tc, ExitStack() as ctx:
        const = ctx.enter_context(tc.tile_pool(name="const", bufs=1))
        dramp = ctx.enter_context(tc.tile_pool(name="dram", bufs=1, space="DRAM"))
        xp_t = dramp.tile([N // 2, ROWF], bf16)

        wlint = const.tile([IN_F, 132], bf16)
        nc.sync.dma_start(wlint[:], wlin_d.ap())
        dext_t = const.tile([IN_F + 1, HEADS], bf16)
        nc.sync.dma_start(dext_t[:], dext_d.ap())
        crep_t = const.tile([128, HEADS * EDGE_F], bf16)
        nc.sync.dma_start(crep_t[:], crep_d.ap())
        wrese_t = const.tile([IN_F + 1, 128], bf16)
        nc.sync.dma_start(wrese_t[:], wrese_d.ap())
        xTown = const.tile([IN_F + 1, WNODES], bf16)
        nc.sync.dma_start(xTown[:], xT_d.ap()[:, 0:WNODES])
        l8t = const.tile([128, SUMT * 8], bf16)
        nc.sync.dma_start(l8t[:], l8_d.ap())
        atb = const.tile([128, NW * HEADS], bf16)

        # ---- pass-0: gather table ([25000, 384] bf16 pair rows) + a_t ----
        NBLK = (N + 127) // 128          # 391 node blocks of 128
        GB = 8                           # blocks per batched table write
        SLABW = 12544                    # 98 blocks per slab
        with tc.tile_pool(name="p0slab", bufs=2) as slabp, \
             tc.tile_pool(name="p0", bufs=3) as p0, \
             tc.tile_pool(name="p0ps", bufs=4, space="PSUM") as p0ps:
            xp_flat = xp_t[:]            # [25000, 384]
            nslab = (N + SLABW - 1) // SLABW
            for sl in range(nslab):
                c0 = sl * SLABW
                cw = min(SLABW, N - c0)
                slab = slabp.tile([IN_F, SLABW], bf16, tag="slab")
                nc.sync.dma_start(slab[:, :cw], xT_d.ap()[0:IN_F, c0:c0 + cw])
                b0 = c0 // 128
                bn = (cw + 127) // 128
                for bg in range(b0, b0 + bn, GB):
                    gn = min(GB, b0 + bn - bg)
                    stage = p0.tile([128, GB * 132], bf16, tag="stage")
                    for k in range(gn):
                        b = bg + k
                        nb = min(128, N - b * 128)
                        lo = b * 128 - c0
                        if nb < 128:
                            nc.vector.memset(stage[:, k * 132:(k + 1) * 132], 0.0)
                        ps = p0ps.tile([128, 132], f32, tag="ps")
                        nc.tensor.matmul(ps[:nb, :], slab[:, lo:lo + nb],
                                         wlint[:], start=True, stop=True)
                        if k % 2 == 0:
                            nc.scalar.copy(stage[:nb, k * 132:(k + 1) * 132], ps[:nb, :])
                        else:
                            nc.vector.tensor_copy(stage[:nb, k * 132:(k + 1) * 132], ps[:nb, :])
                    gfull = gn
                    if bg + gn == NBLK and N % 128 != 0:
                        gfull = gn - 1
                    for par in range(2):
                        src = stage[:].rearrange("(r a) c -> a r c", a=2)[par] \
                                      .rearrange("r (k c) -> r k c", c=132)
                        if gfull > 0:
                            dst_xp = xp_flat[64 * bg: 64 * (bg + gfull),
                                             128 * par: 128 * par + 128] \
                                .rearrange("(k r) f -> r k f", k=gfull)
                            nc.sync.dma_start(dst_xp, src[:, :gfull, 0:128])
                            dst_as = xp_flat[64 * bg: 64 * (bg + gfull),
                                             256 + HEADS * par: 256 + HEADS * (par + 1)] \
                                .rearrange("(k r) h -> r k h", k=gfull)
                            nc.sync.dma_start(dst_as, src[:, :gfull, 128:132])
                        if gfull < gn:
                            b = bg + gfull
                            rows = (N - b * 128) // 2     # pair rows in partial block
                            r0 = 64 * b
                            nc.sync.dma_start(
                                xp_flat[r0: r0 + rows, 128 * par: 128 * par + 128],
                                src[:rows, gfull, 0:128])
                            nc.sync.dma_start(
                                xp_flat[r0: r0 + rows,
                                        256 + HEADS * par: 256 + HEADS * (par + 1)],
                                src[:rows, gfull, 128:132])
            for w in range(NW):
                ps2 = p0ps.tile([128, HEADS], f32, tag="ps2")
                nc.tensor.matmul(ps2[:], xTown[:, w * 128:(w + 1) * 128], dext_t[:],
                                 start=True, stop=True)
                nc.scalar.copy(atb[:, w * HEADS:(w + 1) * HEADS], ps2[:])

        # ---- main loop ----
        with tc.tile_pool(name="xsp", bufs=2) as xsp, \
             tc.tile_pool(name="eap", bufs=3) as eap, \
             tc.tile_pool(name="idxp", bufs=3) as idxp, \
             tc.tile_pool(name="scr", bufs=2) as scr, \
             tc.tile_pool(name="sml", bufs=2) as sml, \
             tc.tile_pool(name="rhsp", bufs=2) as rhsp, \
             tc.tile_pool(name="nap", bufs=2) as nap, \
             tc.tile_pool(name="outp", bufs=3) as outp, \
             tc.tile_pool(name="mps", bufs=2, space="PSUM") as mps:

            qrr = 0
            wchunks = {}
            for ch in chunks:
                wchunks.setdefault(ch[0], []).append(ch)

            for w in range(NW):
                res_ps = mps.tile([128, 128], f32, tag="res")
                nc.tensor.matmul(res_ps[:], xTown[:, w * 128:(w + 1) * 128],
                                 wrese_t[:], start=True, stop=True)
                num_acc = nap.tile([128, 132], f32, tag="num")
                first = True
                for (_, t0, t1) in wchunks[w]:
                    tcn = t1 - t0
                    scol = int(TOFF[w]) + t0
                    icol = scol * 8

                    idxc = idxp.tile([128, TC_TILES * 8], i16, tag="idxc")
                    nc.sync.dma_start(idxc[:, :tcn * 8],
                                      idx_d.ap()[:, icol: icol + tcn * 8])
                    xs = xsp.tile([128, TC_TILES, ROWF], bf16, tag="xs")
                    tpos = 0
                    while tpos < tcn:
                        tn = min(GCALL_TILES, tcn - tpos)
                        nc.gpsimd.dma_gather(
                            xs[:, tpos:tpos + tn, :], xp_t[:],
                            idxc[:, tpos * 8:(tpos + tn) * 8],
                            tn * 128, tn * 128, ROWF, single_packet=True,
                            queue_num=qrr % 4)
                        qrr += 1
                        tpos += tn

                    eat = eap.tile([128, TC_TILES * EDGE_F], bf16, tag="eat")
                    nc.sync.dma_start(eat[:, :tcn * EDGE_F],
                                      ea_d.ap()[:, scol * EDGE_F: (scol + tcn) * EDGE_F])

                    # a_e: grouped product + tree reduce, all bf16 STT (4x)
                    prode = scr.tile([128, TC_TILES * HEADS * EDGE_F], bf16, tag="prode")
                    ea_bc = eat[:, :tcn * EDGE_F] \
                        .rearrange("p (t k) -> p t k", t=tcn) \
                        .rearrange("p t (a k) -> p t a k", a=1) \
                        .broadcast_to([128, tcn, HEADS, EDGE_F])
                    crep_bc = crep_t[:].rearrange("p (a f) -> p a f", a=1) \
                        .broadcast_to([128, tcn, HEADS * EDGE_F]) \
                        .rearrange("p t (h k) -> p t h k", h=HEADS)
                    prode_v = prode[:, :tcn * HEADS * EDGE_F] \
                        .rearrange("p (t h k) -> p t h k", t=tcn, h=HEADS)
                    nc.vector.tensor_tensor(prode_v, ea_bc, crep_bc, op=AL.mult)
                    kk = EDGE_F
                    while kk > 1:
                        half = kk // 2
                        nc.vector.tensor_tensor(
                            prode_v[:, :, :, 0:half], prode_v[:, :, :, 0:half],
                            prode_v[:, :, :, kk - half:kk], op=AL.add)
                        kk -= half

                    # u8 = a_s(row) + ze + atb + L8 ; lrelu; [128, t, 2, 4]
                    ze_b = prode_v[:, :, :, 0:1] \
                        .rearrange("p t h a -> p t (h a)") \
                        .rearrange("p t (a h) -> p t a h", a=1) \
                        .broadcast_to([128, tcn, 2, HEADS])
                    atb_b = atb[:, w * HEADS:(w + 1) * HEADS] \
                        .rearrange("p (a b h) -> p a b h", a=1, b=1) \
                        .broadcast_to([128, tcn, 2, HEADS])
                    l8_b = l8t[:, scol * 8:(scol + tcn) * 8] \
                        .rearrange("p (t a h) -> p t a h", t=tcn, a=2)
                    as8 = xs[:, :tcn, 256:264].rearrange("p t (a h) -> p t a h", a=2)
                    u8 = sml.tile([128, TC_TILES * 8], bf16, tag="u8")
                    u8_v = u8[:, :tcn * 8].rearrange("p (t a h) -> p t a h", t=tcn, a=2)
                    nc.vector.tensor_tensor(u8_v, l8_b, atb_b, op=AL.add)
                    nc.vector.tensor_tensor(u8_v, u8_v, ze_b, op=AL.add)
                    nc.vector.tensor_tensor(u8_v, u8_v, as8, op=AL.add)
                    u8_f = u8[:, :tcn * 8]
                    nc.vector.scalar_tensor_tensor(u8_f, u8_f, NEG_SLOPE, u8_f,
                                                   op0=AL.mult, op1=AL.max)

                    # exp with expanded output -> rhs[p, t, 2, 4, 33]
                    rhs = rhsp.tile([128, TC_TILES, 2, 132], bf16, tag="rhs")
                    rhs_e = rhs[:, :tcn, :, :].rearrange("p t a (h f) -> p t a h f", h=HEADS)
                    u8_bc = u8_v.rearrange("p t a (h f) -> p t a h f", f=1) \
                        .broadcast_to([128, tcn, 2, HEADS, 33])
                    nc.scalar.activation(rhs_e, u8_bc,
                                         mybir.ActivationFunctionType.Exp)

                    # msg: rhs[..., h, 0:32] *= xs pair halves (STT 4x, in place)
                    msg_v = rhs_e[:, :, :, :, 0:32]
                    xs_v = xs[:, :tcn, 0:256] \
                        .rearrange("p t (a h f) -> p t a h f", a=2, h=HEADS)
                    nc.vector.tensor_tensor(msg_v, msg_v, xs_v, op=AL.mult)

                    # fold slots: [128, 2t, 132] -> num_acc (TT adds, bf16 2x)
                    flat = rhs[:, :tcn, :, :].rearrange("p t h f -> p (t h) f")
                    n = 2 * tcn
                    while n > 2:
                        k = n // 2
                        nc.vector.tensor_tensor(
                            flat[:, 0:k, :], flat[:, 0:k, :],
                            flat[:, n - k:n, :], op=AL.add)
                        n -= k
                    if first:
                        nc.vector.tensor_tensor(num_acc[:], flat[:, 0, :],
                                                flat[:, n - 1, :], op=AL.add)
                        first = False
                    else:
                        nc.vector.tensor_tensor(flat[:, 0, :], flat[:, 0, :],
                                                flat[:, n - 1, :], op=AL.add)
                        nc.vector.tensor_tensor(num_acc[:], num_acc[:], flat[:, 0, :],
                                                op=AL.add)

                # ---- window close (num cols h*33+f, denom col h*33+32) ----
                nv = num_acc[:].rearrange("p (h f) -> p h f", h=HEADS)
                dn_src = nv[:, :, 32:33].rearrange("p h a -> p (h a)")
                rec = outp.tile([128, HEADS], f32, tag="rec")
                nc.vector.reciprocal(rec[:], dn_src)
                outw = outp.tile([128, 128], f32, tag="outw")
                outw_v = outw[:].rearrange("p (h f) -> p h f", h=HEADS)
                rec_bc = rec[:].rearrange("p (h a) -> p h a", a=1) \
                               .broadcast_to([128, HEADS, OUT_F])
                nc.vector.tensor_tensor(outw_v, nv[:, :, 0:32], rec_bc, op=AL.mult)
                out2 = outp.tile([128, 128], f32, tag="out2")
                nc.vector.tensor_tensor(out2[:], outw[:], res_ps[:], op=AL.add)
                nc.sync.dma_start(out_d.ap()[w * 128:(w + 1) * 128, :], out2[:])

    nc.compile()
    return nc


def kernel(**inputs):
    from concourse.bass_utils import run_bass_kernel_spmd

    args = {k: np.asarray(v) for k, v in inputs.items()}
    common, per_core = _host_preprocess(
        args["x"], args["edge_index"], args["edge_attr"], args["W_lin"],
        args["w_s"], args["b_s"], args["w_t"], args["b_t"], args["W_edge"],
        args["w_e"], args["b_e"], args["W_res"], args["bias"])

    nc = _build_program(common)

    in_maps = []
    for c in range(NCORES):
        pc = per_core[c]
        in_maps.append({
            "xT": pc["xT"], "idx16": pc["idx16"], "l8": pc["l8"], "ea": pc["ea"],
            "wlind": common["wlind"], "dext": common["dext"],
            "crep": common["crep"], "wrese": common["wrese"],
        })

    res = run_bass_kernel_spmd(nc, in_maps, list(range(NCORES)),
                               trace=bool(os.environ.get("GAT_TRACE")),
                               tmpdir=os.environ.get("GAT_TMPDIR"))
    if os.environ.get("GAT_TRACE"):
        print(f"HW exec time: {res.exec_time_ns} ns")

    out = np.empty((N, HEADS * OUT_F), np.float32)
    for c in range(NCORES):
        out[per_core[c]["perm_owned"]] = res.results[c]["out"][:NODES_PC]
    return out


# revision 13
# speedup vs baseline: 1.1125x; 1.1125x over previous
"""GAT layer (gnn_message_passing) on 8 trn2 NeuronCores.

Strategy (dst-sharded, no collectives), v2:
- Each core owns a contiguous 1/8 slice of target nodes; host buckets edges by
  dst core. Within a core, owned nodes are sorted by in-degree (descending) and
  grouped into 128-node windows; node -> SBUF partition, its in-edges occupy
  "slot columns" t=0..deg-1 of that partition.
- Per edge slot, a 768B bf16 row [xp[2j] | xp[2j+1] | a_s[2j] | a_s[2j+1] |
  pad] is fetched with SWDGE dma_gather (idx = perm_pos(src)>>1, int16).
  Gather calls are kept small (<=256 descs) and round-robin over 4 queues to
  stay under the SWDGE ring-stall knee.
- Parity (which half of the pair is the real src) and slot validity are folded
  into an additive logit mask L8 (0 or -100): exp(u - 100) == 0, so no select
  masks and no wasted multiplies.
- Attention logits u = a_s(row) + a_t+biases(pass-0 col) + a_e(edge-attr
  grouped reduce) computed in [128, t, 2, 4] bf16; leaky-relu via STT.
- ACT (Scalar) engine computes exp with a pre-EXPANDED output written straight
  into the rhs buffer in an interleaved [head, 32 feat + 1 denom] = 132-col
  layout; DVE then multiplies rhs by xs in place (STT, 4x bf16 mode) and
  pairwise-folds slots (STT adds). Numerator + denominator come out of one
  fold. Residual x @ W_res.T + bias via ones-row-extended matmul (PE, bf16).
"""
import os
import sys
from contextlib import ExitStack

sys.path.insert(0, "/opt/trn_rl_repo")

import numpy as np

N, E = 50000, 1600000
IN_F, EDGE_F, HEADS, OUT_F = 64, 16, 4, 32
NEG_SLOPE = 0.2
NCORES = 8
NODES_PC = N // NCORES            # 6250
NW = (NODES_PC + 127) // 128      # 49 windows/core
WNODES = NW * 128                 # 6272 (last window partially real)
TC_TILES = 48                     # compute-chunk size in 128-slot tiles
GCALL_TILES = 2                   # tiles per dma_gather call (256 descs)
ROWF = 384                        # bf16 row: 256 xp-pair + 8 a_s + 120 pad
# exp(LMASK) must stay nonzero in bf16 so empty segments keep a nonzero
# denominator (no clamp needed): exp(-80) ~ 1.8e-35 > bf16 min normal.
LMASK = -80.0


def _bf16(a):
    import ml_dtypes
    return np.ascontiguousarray(np.asarray(a, np.float32).astype(ml_dtypes.bfloat16))


def _host_preprocess(x, edge_index, edge_attr, W_lin, w_s, b_s, w_t, b_t,
                     W_edge, w_e, b_e, W_res, bias):
    """Pure index/layout work + weight folding. Returns (common, per_core)."""
    src = edge_index[0].astype(np.int64)
    dst = edge_index[1].astype(np.int64)
    deg = np.bincount(dst, minlength=N)

    # ---- weight folding (weights only; standard operator fusion) ----
    wlinT = np.ascontiguousarray(W_lin.T)                      # [64, 128]
    C = (W_edge.reshape(HEADS, OUT_F, EDGE_F) * w_e[None, :, None]).sum(1)  # [4,16]
    crep = np.tile(C.reshape(-1)[None, :], (128, 1)).astype(np.float32)    # [128,64]
    D = (W_lin.reshape(HEADS, OUT_F, IN_F) * w_t[None, :, None]).sum(1).T  # [64,4]
    b_total = float(b_s) + float(b_t) + float(b_e)
    dext = np.vstack([D, np.full((1, HEADS), b_total, np.float32)]).astype(np.float32)
    Dws = (W_lin.reshape(HEADS, OUT_F, IN_F) * w_s[None, :, None]).sum(1).T  # [64,4]
    wlind = np.concatenate([wlinT.astype(np.float32), Dws.astype(np.float32)],
                           axis=1)                                          # [64,132]
    wrese = np.vstack([W_res.T, bias[None, :]]).astype(np.float32)         # [65,128]

    # ---- per-core schedules (common T_w across cores) ----
    cores = []
    for c in range(NCORES):
        lo = c * NODES_PC
        owned = np.arange(lo, lo + NODES_PC)
        dc = deg[owned]
        order = np.argsort(-dc, kind="stable")
        perm_owned = owned[order]
        degs_sorted = dc[order]
        tw = np.maximum(degs_sorted[::128][:NW], 1).astype(np.int64)
        cores.append(dict(perm_owned=perm_owned, tw=tw))

    T_w = np.max(np.stack([cc["tw"] for cc in cores]), axis=0)  # [NW]
    TOFF = np.concatenate([[0], np.cumsum(T_w)])                # slot col offsets
    SUMT = int(TOFF[-1])

    chunks = []           # (w, t0, t1)
    for w in range(NW):
        t = 0
        while t < T_w[w]:
            t1 = min(t + TC_TILES, int(T_w[w]))
            chunks.append((w, t, t1))
            t = t1

    per_core = []
    for c in range(NCORES):
        cc = cores[c]
        perm_owned = cc["perm_owned"]
        rest = np.setdiff1d(np.arange(N), perm_owned, assume_unique=True)
        perm = np.concatenate([perm_owned, rest])
        perm_pos = np.empty(N, np.int64)
        perm_pos[perm] = np.arange(N)

        emask = (dst >= c * NODES_PC) & (dst < (c + 1) * NODES_PC)
        e_ids = np.nonzero(emask)[0]
        d_loc = perm_pos[dst[e_ids]]                 # 0..6249
        eorder = np.argsort(d_loc, kind="stable")
        e_s = e_ids[eorder]
        ds = d_loc[eorder]
        starts = np.searchsorted(ds, np.arange(NODES_PC))
        t_of = np.arange(len(ds)) - starts[ds]
        w_of = ds // 128
        p_of = ds % 128
        col = TOFF[w_of] + t_of

        src_rel = perm_pos[src[e_s]]
        par = (src_rel & 1).astype(np.int64)

        idx_slot = np.zeros((128, SUMT), np.int16)
        idx_slot[p_of, col] = (src_rel >> 1).astype(np.int16)

        # additive logit mask: [128, SUMT, 2(parity half), 4(heads)]
        l8 = np.full((128, SUMT, 2, HEADS), np.float32(LMASK), np.float32)
        l8[p_of, col, par] = 0.0

        ea_slot = np.zeros((128, SUMT, EDGE_F), np.float32)
        ea_slot[p_of, col] = edge_attr[e_s]

        # idx wrapped in 16 partitions (per window), replicated x8
        idx16 = np.zeros((128, SUMT * 8), np.int16)
        for w in range(NW):
            t0, t1 = int(TOFF[w]), int(TOFF[w + 1])
            flat = idx_slot[:, t0:t1].T.reshape(-1)
            wrapped = flat.reshape(-1, 16).T          # [16, T_w*8]
            idx16[:, t0 * 8: t1 * 8] = np.tile(wrapped, (8, 1))

        xT_ext = np.empty((IN_F + 1, N), np.float32)
        xT_ext[:IN_F] = x[perm].T
        xT_ext[IN_F] = 1.0

        per_core.append(dict(
            xT=_bf16(xT_ext),
            idx16=idx16,
            l8=_bf16(l8.reshape(128, SUMT * 8)),
            ea=_bf16(ea_slot.reshape(128, SUMT * EDGE_F)),
            perm_owned=perm_owned,
        ))

    common = dict(T_w=T_w, TOFF=TOFF, SUMT=SUMT, chunks=chunks,
                  wlind=_bf16(wlind), dext=_bf16(dext), crep=_bf16(crep),
                  wrese=_bf16(wrese))
    return common, per_core


def _build_program(common):
    import concourse.bass as bass
    import concourse.tile as tile
    from concourse import bacc, mybir

    f32 = mybir.dt.float32
    bf16 = mybir.dt.bfloat16
    i16 = mybir.dt.int16
    AL = mybir.AluOpType
    SUMT = common["SUMT"]
    T_w, TOFF, chunks = common["T_w"], common["TOFF"], common["chunks"]

    nc = bacc.Bacc("TRN2", target_bir_lowering=False, debug=False,
                   num_devices=NCORES, num_swdge_queues=4)

    xT_d = nc.dram_tensor("xT", [IN_F + 1, N], bf16, kind="ExternalInput")
    idx_d = nc.dram_tensor("idx16", [128, SUMT * 8], i16, kind="ExternalInput")
    l8_d = nc.dram_tensor("l8", [128, SUMT * 8], bf16, kind="ExternalInput")
    ea_d = nc.dram_tensor("ea", [128, SUMT * EDGE_F], bf16, kind="ExternalInput")
    wlin_d = nc.dram_tensor("wlind", [IN_F, 132], bf16, kind="ExternalInput")
    dext_d = nc.dram_tensor("dext", [IN_F + 1, HEADS], bf16, kind="ExternalInput")
    crep_d = nc.dram_tensor("crep", [128, HEADS * EDGE_F], bf16, kind="ExternalInput")
    wrese_d = nc.dram_tensor("wrese", [IN_F + 1, 128], bf16, kind="ExternalInput")
    out_d = nc.dram_tensor("out", [WNODES, 128], f32, kind="ExternalOutput")

    with tile.TileContext(nc) as tc, ExitStack() as ctx:
        const = ctx.enter_context(tc.tile_pool(name="const", bufs=1))
        dramp = ctx.enter_context(tc.tile_pool(name="dram", bufs=1, space="DRAM"))
        xp_t = dramp.tile([N // 2, ROWF], bf16)

        wlint = const.tile([IN_F, 132], bf16)
        nc.sync.dma_start(wlint[:], wlin_d.ap())
        dext_t = const.tile([IN_F + 1, HEADS], bf16)
        nc.sync.dma_start(dext_t[:], dext_d.ap())
        crep_t = const.tile([128, HEADS * EDGE_F], bf16)
        nc.sync.dma_start(crep_t[:], crep_d.ap())
        wrese_t = const.tile([IN_F + 1, 128], bf16)
        nc.sync.dma_start(wrese_t[:], wrese_d.ap())
        xTown = const.tile([IN_F + 1, WNODES], bf16)
        nc.sync.dma_start(xTown[:], xT_d.ap()[:, 0:WNODES])
        l8t = const.tile([128, SUMT * 8], bf16)
        nc.sync.dma_start(l8t[:], l8_d.ap())
        atb = const.tile([128, NW * HEADS], bf16)

        # ---- pass-0: gather table ([25000, 384] bf16 pair rows) + a_t ----
        NBLK = (N + 127) // 128          # 391 node blocks of 128
        GB = 8                           # blocks per batched table write
        SLABW = 12544                    # 98 blocks per slab
        with tc.tile_pool(name="p0slab", bufs=2) as slabp, \
             tc.tile_pool(name="p0", bufs=3) as p0, \
             tc.tile_pool(name="p0ps", bufs=4, space="PSUM") as p0ps:
            xp_flat = xp_t[:]            # [25000, 384]
            nslab = (N + SLABW - 1) // SLABW
            for sl in range(nslab):
                c0 = sl * SLABW
                cw = min(SLABW, N - c0)
                slab = slabp.tile([IN_F, SLABW], bf16, tag="slab")
                nc.sync.dma_start(slab[:, :cw], xT_d.ap()[0:IN_F, c0:c0 + cw])
                b0 = c0 // 128
                bn = (cw + 127) // 128
                for bg in range(b0, b0 + bn, GB):
                    gn = min(GB, b0 + bn - bg)
                    stage = p0.tile([128, GB * 132], bf16, tag="stage")
                    for k in range(gn):
                        b = bg + k
                        nb = min(128, N - b * 128)
                        lo = b * 128 - c0
                        if nb < 128:
                            nc.vector.memset(stage[:, k * 132:(k + 1) * 132], 0.0)
                        ps = p0ps.tile([128, 132], f32, tag="ps")
                        nc.tensor.matmul(ps[:nb, :], slab[:, lo:lo + nb],
                                         wlint[:], start=True, stop=True)
                        if k % 2 == 0:
                            nc.scalar.copy(stage[:nb, k * 132:(k + 1) * 132], ps[:nb, :])
                        else:
                            nc.vector.tensor_copy(stage[:nb, k * 132:(k + 1) * 132], ps[:nb, :])
                    gfull = gn
                    if bg + gn == NBLK and N % 128 != 0:
                        gfull = gn - 1
                    for par in range(2):
                        src = stage[:].rearrange("(r a) c -> a r c", a=2)[par] \
                                      .rearrange("r (k c) -> r k c", c=132)
                        if gfull > 0:
                            dst_xp = xp_flat[64 * bg: 64 * (bg + gfull),
                                             128 * par: 128 * par + 128] \
                                .rearrange("(k r) f -> r k f", k=gfull)
                            nc.sync.dma_start(dst_xp, src[:, :gfull, 0:128])
                            dst_as = xp_flat[64 * bg: 64 * (bg + gfull),
                                             256 + HEADS * par: 256 + HEADS * (par + 1)] \
                                .rearrange("(k r) h -> r k h", k=gfull)
                            nc.sync.dma_start(dst_as, src[:, :gfull, 128:132])
                        if gfull < gn:
                            b = bg + gfull
                            rows = (N - b * 128) // 2     # pair rows in partial block
                            r0 = 64 * b
                            nc.sync.dma_start(
                                xp_flat[r0: r0 + rows, 128 * par: 128 * par + 128],
                                src[:rows, gfull, 0:128])
                            nc.sync.dma_start(
                                xp_flat[r0: r0 + rows,
                                        256 + HEADS * par: 256 + HEADS * (par + 1)],
                                src[:rows, gfull, 128:132])
            for w in range(NW):
                ps2 = p0ps.tile([128, HEADS], f32, tag="ps2")
                nc.tensor.matmul(ps2[:], xTown[:, w * 128:(w + 1) * 128], dext_t[:],
                                 start=True, stop=True)
                nc.scalar.copy(atb[:, w * HEADS:(w + 1) * HEADS], ps2[:])

        # ---- main loop ----
        with tc.tile_pool(name="xsp", bufs=2) as xsp, \
             tc.tile_pool(name="eap", bufs=3) as eap, \
             tc.tile_pool(name="idxp", bufs=3) as idxp, \
             tc.tile_pool(name="scr", bufs=2) as scr, \
             tc.tile_pool(name="sml", bufs=2) as sml, \
             tc.tile_pool(name="rhsp", bufs=2) as rhsp, \
             tc.tile_pool(name="nap", bufs=2) as nap, \
             tc.tile_pool(name="outp", bufs=3) as outp, \
             tc.tile_pool(name="mps", bufs=2, space="PSUM") as mps:

            qrr = 0
            wchunks = {}
            for ch in chunks:
                wchunks.setdefault(ch[0], []).append(ch)

            for w in range(NW):
                res_ps = mps.tile([128, 128], f32, tag="res")
                nc.tensor.matmul(res_ps[:], xTown[:, w * 128:(w + 1) * 128],
                                 wrese_t[:], start=True, stop=True)
                num_acc = nap.tile([128, 132], f32, tag="num")
                first = True
                for (_, t0, t1) in wchunks[w]:
                    tcn = t1 - t0
                    scol = int(TOFF[w]) + t0
                    icol = scol * 8

                    idxc = idxp.tile([128, TC_TILES * 8], i16, tag="idxc")
                    nc.sync.dma_start(idxc[:, :tcn * 8],
                                      idx_d.ap()[:, icol: icol + tcn * 8])
                    xs = xsp.tile([128, TC_TILES, ROWF], bf16, tag="xs")
                    tpos = 0
                    while tpos < tcn:
                        tn = min(GCALL_TILES, tcn - tpos)
                        nc.gpsimd.dma_gather(
                            xs[:, tpos:tpos + tn, :], xp_t[:],
                            idxc[:, tpos * 8:(tpos + tn) * 8],
                            tn * 128, tn * 128, ROWF, single_packet=True,
                            queue_num=qrr % 4)
                        qrr += 1
                        tpos += tn

                    eat = eap.tile([128, TC_TILES * EDGE_F], bf16, tag="eat")
                    nc.sync.dma_start(eat[:, :tcn * EDGE_F],
                                      ea_d.ap()[:, scol * EDGE_F: (scol + tcn) * EDGE_F])

                    # a_e: grouped product + tree reduce, all bf16 STT (4x)
                    prode = scr.tile([128, TC_TILES * HEADS * EDGE_F], bf16, tag="prode")
                    ea_bc = eat[:, :tcn * EDGE_F] \
                        .rearrange("p (t k) -> p t k", t=tcn) \
                        .rearrange("p t (a k) -> p t a k", a=1) \
                        .broadcast_to([128, tcn, HEADS, EDGE_F])
                    crep_bc = crep_t[:].rearrange("p (a f) -> p a f", a=1) \
                        .broadcast_to([128, tcn, HEADS * EDGE_F]) \
                        .rearrange("p t (h k) -> p t h k", h=HEADS)
                    prode_v = prode[:, :tcn * HEADS * EDGE_F] \
                        .rearrange("p (t h k) -> p t h k", t=tcn, h=HEADS)
                    nc.vector.tensor_tensor(prode_v, ea_bc, crep_bc, op=AL.mult)
                    kk = EDGE_F
                    while kk > 1:
                        half = kk // 2
                        nc.vector.tensor_tensor(
                            prode_v[:, :, :, 0:half], prode_v[:, :, :, 0:half],
                            prode_v[:, :, :, kk - half:kk], op=AL.add)
                        kk -= half

                    # u8 = a_s(row) + ze + atb + L8 ; lrelu; [128, t, 2, 4]
                    ze_b = prode_v[:, :, :, 0:1] \
                        .rearrange("p t h a -> p t (h a)") \
                        .rearrange("p t (a h) -> p t a h", a=1) \
                        .broadcast_to([128, tcn, 2, HEADS])
                    atb_b = atb[:, w * HEADS:(w + 1) * HEADS] \
                        .rearrange("p (a b h) -> p a b h", a=1, b=1) \
                        .broadcast_to([128, tcn, 2, HEADS])
                    l8_b = l8t[:, scol * 8:(scol + tcn) * 8] \
                        .rearrange("p (t a h) -> p t a h", t=tcn, a=2)
                    as8 = xs[:, :tcn, 256:264].rearrange("p t (a h) -> p t a h", a=2)
                    u8 = sml.tile([128, TC_TILES * 8], bf16, tag="u8")
                    u8_v = u8[:, :tcn * 8].rearrange("p (t a h) -> p t a h", t=tcn, a=2)
                    nc.vector.tensor_tensor(u8_v, l8_b, atb_b, op=AL.add)
                    nc.vector.tensor_tensor(u8_v, u8_v, ze_b, op=AL.add)
                    nc.vector.tensor_tensor(u8_v, u8_v, as8, op=AL.add)
                    u8_f = u8[:, :tcn * 8]
                    nc.vector.scalar_tensor_tensor(u8_f, u8_f, NEG_SLOPE, u8_f,
                                                   op0=AL.mult, op1=AL.max)

                    # exp with expanded output -> rhs[p, t, 2, 4, 33]
                    rhs = rhsp.tile([128, TC_TILES, 2, 132], bf16, tag="rhs")
                    rhs_e = rhs[:, :tcn, :, :].rearrange("p t a (h f) -> p t a h f", h=HEADS)
                    u8_bc = u8_v.rearrange("p t a (h f) -> p t a h f", f=1) \
                        .broadcast_to([128, tcn, 2, HEADS, 33])
                    nc.scalar.activation(rhs_e, u8_bc,
                                         mybir.ActivationFunctionType.Exp)

                    # msg: rhs[..., h, 0:32] *= xs pair halves (STT 4x, in place)
                    msg_v = rhs_e[:, :, :, :, 0:32]
                    xs_v = xs[:, :tcn, 0:256] \
                        .rearrange("p t (a h f) -> p t a h f", a=2, h=HEADS)
                    nc.vector.tensor_tensor(msg_v, msg_v, xs_v, op=AL.mult)

                    # fold slots: [128, 2t, 132] -> num_acc (TT adds, bf16 2x)
                    flat = rhs[:, :tcn, :, :].rearrange("p t h f -> p (t h) f")
                    n = 2 * tcn
                    while n > 2:
                        k = n // 2
                        nc.vector.tensor_tensor(
                            flat[:, 0:k, :], flat[:, 0:k, :],
                            flat[:, n - k:n, :], op=AL.add)
                        n -= k
                    if first:
                        nc.vector.tensor_tensor(num_acc[:], flat[:, 0, :],
                                                flat[:, n - 1, :], op=AL.add)
                        first = False
                    else:
                        nc.vector.tensor_tensor(flat[:, 0, :], flat[:, 0, :],
                                                flat[:, n - 1, :], op=AL.add)
                        nc.vector.tensor_tensor(num_acc[:], num_acc[:], flat[:, 0, :],
                                                op=AL.add)

                # ---- window close (num cols h*33+f, denom col h*33+32) ----
                nv = num_acc[:].rearrange("p (h f) -> p h f", h=HEADS)
                dn_src = nv[:, :, 32:33].rearrange("p h a -> p (h a)")
                rec = outp.tile([128, HEADS], f32, tag="rec")
                nc.vector.reciprocal(rec[:], dn_src)
                outw = outp.tile([128, 128], f32, tag="outw")
                outw_v = outw[:].rearrange("p (h f) -> p h f", h=HEADS)
                rec_bc = rec[:].rearrange("p (h a) -> p h a", a=1) \
                               .broadcast_to([128, HEADS, OUT_F])
                nc.vector.tensor_tensor(outw_v, nv[:, :, 0:32], rec_bc, op=AL.mult)
                out2 = outp.tile([128, 128], f32, tag="out2")
                nc.vector.tensor_tensor(out2[:], outw[:], res_ps[:], op=AL.add)
                nc.sync.dma_start(out_d.ap()[w * 128:(w + 1) * 128, :], out2[:])

    nc.compile()
    return nc


def kernel(**inputs):
    from concourse.bass_utils import run_bass_kernel_spmd

    args = {k: np.asarray(v) for k, v in inputs.items()}
    common, per_core = _host_preprocess(
        args["x"], args["edge_index"], args["edge_attr"], args["W_lin"],
        args["w_s"], args["b_s"], args["w_t"], args["b_t"], args["W_edge"],
        args["w_e"], args["b_e"], args["W_res"], args["bias"])

    nc = _build_program(common)

    in_maps = []
    for c in range(NCORES):
        pc = per_core[c]
        in_maps.append({
            "xT": pc["xT"], "idx16": pc["idx16"], "l8": pc["l8"], "ea": pc["ea"],
            "wlind": common["wlind"], "dext": common["dext"],
            "crep": common["crep"], "wrese": common["wrese"],
        })

    res = run_bass_kernel_spmd(nc, in_maps, list(range(NCORES)),
                               trace=bool(os.environ.get("GAT_TRACE")),
                               tmpdir=os.environ.get("GAT_TMPDIR"))
    if os.environ.get("GAT_TRACE"):
        print(f"HW exec time: {res.exec_time_ns} ns")

    out = np.empty((N, HEADS * OUT_F), np.float32)
    for c in range(NCORES):
        out[per_core[c]["perm_owned"]] = res.results[c]["out"][:NODES_PC]
    return out


# revision 15
# speedup vs baseline: 1.3489x; 1.2125x over previous
"""GAT layer (gnn_message_passing) on 8 trn2 NeuronCores.

Strategy (dst-sharded, no collectives), v2:
- Each core owns a contiguous 1/8 slice of target nodes; host buckets edges by
  dst core. Within a core, owned nodes are sorted by in-degree (descending) and
  grouped into 128-node windows; node -> SBUF partition, its in-edges occupy
  "slot columns" t=0..deg-1 of that partition.
- Per edge slot, a 768B bf16 row [xp[2j] | xp[2j+1] | a_s[2j] | a_s[2j+1] |
  pad] is fetched with SWDGE dma_gather (idx = perm_pos(src)>>1, int16).
  Gather calls are kept small (<=256 descs) and round-robin over 4 queues to
  stay under the SWDGE ring-stall knee.
- Parity (which half of the pair is the real src) and slot validity are folded
  into an additive logit mask L8 (0 or -100): exp(u - 100) == 0, so no select
  masks and no wasted multiplies.
- Attention logits u = a_s(row) + a_t+biases(pass-0 col) + a_e(edge-attr
  grouped reduce) computed in [128, t, 2, 4] bf16; leaky-relu via STT.
- ACT (Scalar) engine computes exp with a pre-EXPANDED output written straight
  into the rhs buffer in an interleaved [head, 32 feat + 1 denom] = 132-col
  layout; DVE then multiplies rhs by xs in place (STT, 4x bf16 mode) and
  pairwise-folds slots (STT adds). Numerator + denominator come out of one
  fold. Residual x @ W_res.T + bias via ones-row-extended matmul (PE, bf16).
"""
import os
import sys
from contextlib import ExitStack

sys.path.insert(0, "/opt/trn_rl_repo")

import numpy as np

N, E = 50000, 1600000
IN_F, EDGE_F, HEADS, OUT_F = 64, 16, 4, 32
NEG_SLOPE = 0.2
NCORES = 8
NODES_PC = N // NCORES            # 6250
NW = (NODES_PC + 127) // 128      # 49 windows/core
WNODES = NW * 128                 # 6272 (last window partially real)
TC_TILES = 40                     # compute-chunk size in 128-slot tiles
GCALL_TILES = 2                   # tiles per dma_gather call (256 descs)
ROWF = 384                        # bf16 row: 256 xp-pair + 8 a_s + 120 pad
# exp(LMASK) must stay nonzero in bf16 so empty segments keep a nonzero
# denominator (no clamp needed): exp(-80) ~ 1.8e-35 > bf16 min normal.
LMASK = -80.0


def _bf16(a):
    import ml_dtypes
    return np.ascontiguousarray(np.asarray(a, np.float32).astype(ml_dtypes.bfloat16))


def _host_preprocess(x, edge_index, edge_attr, W_lin, w_s, b_s, w_t, b_t,
                     W_edge, w_e, b_e, W_res, bias):
    """Pure index/layout work + weight folding. Returns (common, per_core)."""
    src = edge_index[0].astype(np.int64)
    dst = edge_index[1].astype(np.int64)
    deg = np.bincount(dst, minlength=N)

    # ---- weight folding (weights only; standard operator fusion) ----
    wlinT = np.ascontiguousarray(W_lin.T)                      # [64, 128]
    C = (W_edge.reshape(HEADS, OUT_F, EDGE_F) * w_e[None, :, None]).sum(1)  # [4,16]
    crep = np.tile(C.reshape(-1)[None, :], (128, 1)).astype(np.float32)    # [128,64]
    D = (W_lin.reshape(HEADS, OUT_F, IN_F) * w_t[None, :, None]).sum(1).T  # [64,4]
    b_total = float(b_s) + float(b_t) + float(b_e)
    dext = np.vstack([D, np.full((1, HEADS), b_total, np.float32)]).astype(np.float32)
    Dws = (W_lin.reshape(HEADS, OUT_F, IN_F) * w_s[None, :, None]).sum(1).T  # [64,4]
    wlind = np.concatenate([wlinT.astype(np.float32), Dws.astype(np.float32)],
                           axis=1)                                          # [64,132]
    wrese = np.vstack([W_res.T, bias[None, :]]).astype(np.float32)         # [65,128]

    # ---- per-core schedules (common T_w across cores) ----
    cores = []
    for c in range(NCORES):
        lo = c * NODES_PC
        owned = np.arange(lo, lo + NODES_PC)
        dc = deg[owned]
        order = np.argsort(-dc, kind="stable")
        perm_owned = owned[order]
        degs_sorted = dc[order]
        tw = np.maximum(degs_sorted[::128][:NW], 1).astype(np.int64)
        cores.append(dict(perm_owned=perm_owned, tw=tw))

    T_w = np.max(np.stack([cc["tw"] for cc in cores]), axis=0)  # [NW]
    TOFF = np.concatenate([[0], np.cumsum(T_w)])                # slot col offsets
    SUMT = int(TOFF[-1])

    chunks = []           # (w, t0, t1)
    for w in range(NW):
        t = 0
        while t < T_w[w]:
            t1 = min(t + TC_TILES, int(T_w[w]))
            chunks.append((w, t, t1))
            t = t1

    per_core = []
    for c in range(NCORES):
        cc = cores[c]
        perm_owned = cc["perm_owned"]
        rest = np.setdiff1d(np.arange(N), perm_owned, assume_unique=True)
        perm = np.concatenate([perm_owned, rest])
        perm_pos = np.empty(N, np.int64)
        perm_pos[perm] = np.arange(N)

        emask = (dst >= c * NODES_PC) & (dst < (c + 1) * NODES_PC)
        e_ids = np.nonzero(emask)[0]
        d_loc = perm_pos[dst[e_ids]]                 # 0..6249
        eorder = np.argsort(d_loc, kind="stable")
        e_s = e_ids[eorder]
        ds = d_loc[eorder]
        starts = np.searchsorted(ds, np.arange(NODES_PC))
        t_of = np.arange(len(ds)) - starts[ds]
        w_of = ds // 128
        p_of = ds % 128
        col = TOFF[w_of] + t_of

        src_rel = perm_pos[src[e_s]]
        par = (src_rel & 1).astype(np.int64)

        idx_slot = np.zeros((128, SUMT), np.int16)
        idx_slot[p_of, col] = (src_rel >> 1).astype(np.int16)

        # additive logit mask: [128, SUMT, 2(parity half), 4(heads)]
        l8 = np.full((128, SUMT, 2, HEADS), np.float32(LMASK), np.float32)
        l8[p_of, col, par] = 0.0

        ea_slot = np.zeros((128, SUMT, EDGE_F), np.float32)
        ea_slot[p_of, col] = edge_attr[e_s]

        # idx wrapped in 16 partitions (per window), replicated x8
        idx16 = np.zeros((128, SUMT * 8), np.int16)
        for w in range(NW):
            t0, t1 = int(TOFF[w]), int(TOFF[w + 1])
            flat = idx_slot[:, t0:t1].T.reshape(-1)
            wrapped = flat.reshape(-1, 16).T          # [16, T_w*8]
            idx16[:, t0 * 8: t1 * 8] = np.tile(wrapped, (8, 1))

        xT_ext = np.empty((IN_F + 1, N), np.float32)
        xT_ext[:IN_F] = x[perm].T
        xT_ext[IN_F] = 1.0

        per_core.append(dict(
            xT=_bf16(xT_ext),
            idx16=idx16,
            l8=_bf16(l8.reshape(128, SUMT * 8)),
            ea=_bf16(ea_slot.reshape(128, SUMT * EDGE_F)),
            perm_owned=perm_owned,
        ))

    common = dict(T_w=T_w, TOFF=TOFF, SUMT=SUMT, chunks=chunks,
                  wlind=_bf16(wlind), dext=_bf16(dext), crep=_bf16(crep),
                  wrese=_bf16(wrese))
    return common, per_core


def _build_program(common):
    import concourse.bass as bass
    import concourse.tile as tile
    from concourse import bacc, mybir

    f32 = mybir.dt.float32
    bf16 = mybir.dt.bfloat16
    i16 = mybir.dt.int16
    AL = mybir.AluOpType
    SUMT = common["SUMT"]
    T_w, TOFF, chunks = common["T_w"], common["TOFF"], common["chunks"]

    nc = bacc.Bacc("TRN2", target_bir_lowering=False, debug=False,
                   num_devices=NCORES, num_swdge_queues=4)

    xT_d = nc.dram_tensor("xT", [IN_F + 1, N], bf16, kind="ExternalInput")
    idx_d = nc.dram_tensor("idx16", [128, SUMT * 8], i16, kind="ExternalInput")
    l8_d = nc.dram_tensor("l8", [128, SUMT * 8], bf16, kind="ExternalInput")
    ea_d = nc.dram_tensor("ea", [128, SUMT * EDGE_F], bf16, kind="ExternalInput")
    wlin_d = nc.dram_tensor("wlind", [IN_F, 132], bf16, kind="ExternalInput")
    dext_d = nc.dram_tensor("dext", [IN_F + 1, HEADS], bf16, kind="ExternalInput")
    crep_d = nc.dram_tensor("crep", [128, HEADS * EDGE_F], bf16, kind="ExternalInput")
    wrese_d = nc.dram_tensor("wrese", [IN_F + 1, 128], bf16, kind="ExternalInput")
    out_d = nc.dram_tensor("out", [WNODES, 128], f32, kind="ExternalOutput")

    with tile.TileContext(nc) as tc, ExitStack() as ctx:
        const = ctx.enter_context(tc.tile_pool(name="const", bufs=1))
        dramp = ctx.enter_context(tc.tile_pool(name="dram", bufs=1, space="DRAM"))
        xp_t = dramp.tile([N // 2, ROWF], bf16)

        wlint = const.tile([IN_F, 132], bf16)
        nc.sync.dma_start(wlint[:], wlin_d.ap())
        dext_t = const.tile([IN_F + 1, HEADS], bf16)
        nc.sync.dma_start(dext_t[:], dext_d.ap())
        crep_t = const.tile([128, HEADS * EDGE_F], bf16)
        nc.sync.dma_start(crep_t[:], crep_d.ap())
        wrese_t = const.tile([IN_F + 1, 128], bf16)
        nc.sync.dma_start(wrese_t[:], wrese_d.ap())
        xTown = const.tile([IN_F + 1, WNODES], bf16)
        nc.sync.dma_start(xTown[:], xT_d.ap()[:, 0:WNODES])
        l8t = const.tile([128, SUMT * 8], bf16)
        nc.sync.dma_start(l8t[:], l8_d.ap())
        atb = const.tile([128, NW * HEADS], bf16)

        # ---- pass-0: gather table ([25000, 384] bf16 pair rows) + a_t ----
        NBLK = (N + 127) // 128          # 391 node blocks of 128
        GB = 8                           # blocks per batched table write
        SLABW = 12544                    # 98 blocks per slab
        with tc.tile_pool(name="p0slab", bufs=2) as slabp, \
             tc.tile_pool(name="p0", bufs=3) as p0, \
             tc.tile_pool(name="p0ps", bufs=4, space="PSUM") as p0ps:
            xp_flat = xp_t[:]            # [25000, 384]
            nslab = (N + SLABW - 1) // SLABW
            for sl in range(nslab):
                c0 = sl * SLABW
                cw = min(SLABW, N - c0)
                slab = slabp.tile([IN_F, SLABW], bf16, tag="slab")
                nc.sync.dma_start(slab[:, :cw], xT_d.ap()[0:IN_F, c0:c0 + cw])
                b0 = c0 // 128
                bn = (cw + 127) // 128
                for bg in range(b0, b0 + bn, GB):
                    gn = min(GB, b0 + bn - bg)
                    stage = p0.tile([128, GB * 132], bf16, tag="stage")
                    for k in range(gn):
                        b = bg + k
                        nb = min(128, N - b * 128)
                        lo = b * 128 - c0
                        if nb < 128:
                            nc.vector.memset(stage[:, k * 132:(k + 1) * 132], 0.0)
                        ps = p0ps.tile([128, 132], f32, tag="ps")
                        nc.tensor.matmul(ps[:nb, :], slab[:, lo:lo + nb],
                                         wlint[:], start=True, stop=True)
                        if k % 2 == 0:
                            nc.scalar.copy(stage[:nb, k * 132:(k + 1) * 132], ps[:nb, :])
                        else:
                            nc.vector.tensor_copy(stage[:nb, k * 132:(k + 1) * 132], ps[:nb, :])
                    gfull = gn
                    if bg + gn == NBLK and N % 128 != 0:
                        gfull = gn - 1
                    for par in range(2):
                        src = stage[:].rearrange("(r a) c -> a r c", a=2)[par] \
                                      .rearrange("r (k c) -> r k c", c=132)
                        if gfull > 0:
                            dst_xp = xp_flat[64 * bg: 64 * (bg + gfull),
                                             128 * par: 128 * par + 128] \
                                .rearrange("(k r) f -> r k f", k=gfull)
                            nc.sync.dma_start(dst_xp, src[:, :gfull, 0:128])
                            dst_as = xp_flat[64 * bg: 64 * (bg + gfull),
                                             256 + HEADS * par: 256 + HEADS * (par + 1)] \
                                .rearrange("(k r) h -> r k h", k=gfull)
                            nc.sync.dma_start(dst_as, src[:, :gfull, 128:132])
                        if gfull < gn:
                            b = bg + gfull
                            rows = (N - b * 128) // 2     # pair rows in partial block
                            r0 = 64 * b
                            nc.sync.dma_start(
                                xp_flat[r0: r0 + rows, 128 * par: 128 * par + 128],
                                src[:rows, gfull, 0:128])
                            nc.sync.dma_start(
                                xp_flat[r0: r0 + rows,
                                        256 + HEADS * par: 256 + HEADS * (par + 1)],
                                src[:rows, gfull, 128:132])
            for w in range(NW):
                ps2 = p0ps.tile([128, HEADS], f32, tag="ps2")
                nc.tensor.matmul(ps2[:], xTown[:, w * 128:(w + 1) * 128], dext_t[:],
                                 start=True, stop=True)
                nc.scalar.copy(atb[:, w * HEADS:(w + 1) * HEADS], ps2[:])

        # ---- main loop ----
        with tc.tile_pool(name="xsp", bufs=3) as xsp, \
             tc.tile_pool(name="eap", bufs=4) as eap, \
             tc.tile_pool(name="idxp", bufs=4) as idxp, \
             tc.tile_pool(name="scr", bufs=2) as scr, \
             tc.tile_pool(name="sml", bufs=3) as sml, \
             tc.tile_pool(name="rhsp", bufs=2) as rhsp, \
             tc.tile_pool(name="nap", bufs=2) as nap, \
             tc.tile_pool(name="outp", bufs=3) as outp, \
             tc.tile_pool(name="mps", bufs=2, space="PSUM") as mps:

            qrr = 0
            wchunks = {}
            for ch in chunks:
                wchunks.setdefault(ch[0], []).append(ch)

            for w in range(NW):
                res_ps = mps.tile([128, 128], f32, tag="res")
                nc.tensor.matmul(res_ps[:], xTown[:, w * 128:(w + 1) * 128],
                                 wrese_t[:], start=True, stop=True)
                num_acc = nap.tile([128, 132], f32, tag="num")
                first = True
                for (_, t0, t1) in wchunks[w]:
                    tcn = t1 - t0
                    scol = int(TOFF[w]) + t0
                    icol = scol * 8

                    idxc = idxp.tile([128, TC_TILES * 8], i16, tag="idxc")
                    nc.sync.dma_start(idxc[:, :tcn * 8],
                                      idx_d.ap()[:, icol: icol + tcn * 8])
                    xs = xsp.tile([128, TC_TILES, ROWF], bf16, tag="xs")
                    tpos = 0
                    while tpos < tcn:
                        tn = min(GCALL_TILES, tcn - tpos)
                        nc.gpsimd.dma_gather(
                            xs[:, tpos:tpos + tn, :], xp_t[:],
                            idxc[:, tpos * 8:(tpos + tn) * 8],
                            tn * 128, tn * 128, ROWF, single_packet=True,
                            queue_num=qrr % 4)
                        qrr += 1
                        tpos += tn

                    eat = eap.tile([128, TC_TILES * EDGE_F], bf16, tag="eat")
                    nc.sync.dma_start(eat[:, :tcn * EDGE_F],
                                      ea_d.ap()[:, scol * EDGE_F: (scol + tcn) * EDGE_F])

                    # a_e: grouped product + tree reduce, all bf16 STT (4x)
                    prode = scr.tile([128, TC_TILES * HEADS * EDGE_F], bf16, tag="prode")
                    ea_bc = eat[:, :tcn * EDGE_F] \
                        .rearrange("p (t k) -> p t k", t=tcn) \
                        .rearrange("p t (a k) -> p t a k", a=1) \
                        .broadcast_to([128, tcn, HEADS, EDGE_F])
                    crep_bc = crep_t[:].rearrange("p (a f) -> p a f", a=1) \
                        .broadcast_to([128, tcn, HEADS * EDGE_F]) \
                        .rearrange("p t (h k) -> p t h k", h=HEADS)
                    prode_v = prode[:, :tcn * HEADS * EDGE_F] \
                        .rearrange("p (t h k) -> p t h k", t=tcn, h=HEADS)
                    nc.vector.tensor_tensor(prode_v, ea_bc, crep_bc, op=AL.mult)
                    kk = EDGE_F
                    while kk > 1:
                        half = kk // 2
                        nc.vector.tensor_tensor(
                            prode_v[:, :, :, 0:half], prode_v[:, :, :, 0:half],
                            prode_v[:, :, :, kk - half:kk], op=AL.add)
                        kk -= half

                    # u8 = a_s(row) + ze + atb + L8 ; lrelu; [128, t, 2, 4]
                    ze_b = prode_v[:, :, :, 0:1] \
                        .rearrange("p t h a -> p t (h a)") \
                        .rearrange("p t (a h) -> p t a h", a=1) \
                        .broadcast_to([128, tcn, 2, HEADS])
                    atb_b = atb[:, w * HEADS:(w + 1) * HEADS] \
                        .rearrange("p (a b h) -> p a b h", a=1, b=1) \
                        .broadcast_to([128, tcn, 2, HEADS])
                    l8_b = l8t[:, scol * 8:(scol + tcn) * 8] \
                        .rearrange("p (t a h) -> p t a h", t=tcn, a=2)
                    as8 = xs[:, :tcn, 256:264].rearrange("p t (a h) -> p t a h", a=2)
                    u8 = sml.tile([128, TC_TILES * 8], bf16, tag="u8")
                    u8_v = u8[:, :tcn * 8].rearrange("p (t a h) -> p t a h", t=tcn, a=2)
                    nc.vector.tensor_tensor(u8_v, l8_b, atb_b, op=AL.add)
                    nc.vector.tensor_tensor(u8_v, u8_v, ze_b, op=AL.add)
                    nc.vector.tensor_tensor(u8_v, u8_v, as8, op=AL.add)
                    u8_f = u8[:, :tcn * 8]
                    nc.vector.scalar_tensor_tensor(u8_f, u8_f, NEG_SLOPE, u8_f,
                                                   op0=AL.mult, op1=AL.max)

                    # exp with expanded output -> rhs[p, t, 2, 4, 33]
                    rhs = rhsp.tile([128, TC_TILES, 2, 132], bf16, tag="rhs")
                    rhs_e = rhs[:, :tcn, :, :].rearrange("p t a (h f) -> p t a h f", h=HEADS)
                    u8_bc = u8_v.rearrange("p t a (h f) -> p t a h f", f=1) \
                        .broadcast_to([128, tcn, 2, HEADS, 33])
                    nc.scalar.activation(rhs_e, u8_bc,
                                         mybir.ActivationFunctionType.Exp)

                    # msg: rhs[..., h, 0:32] *= xs pair halves (STT 4x, in place)
                    msg_v = rhs_e[:, :, :, :, 0:32]
                    xs_v = xs[:, :tcn, 0:256] \
                        .rearrange("p t (a h f) -> p t a h f", a=2, h=HEADS)
                    nc.vector.tensor_tensor(msg_v, msg_v, xs_v, op=AL.mult)

                    # fold slots: [128, 2t, 132] -> num_acc (TT adds, bf16 2x)
                    flat = rhs[:, :tcn, :, :].rearrange("p t h f -> p (t h) f")
                    n = 2 * tcn
                    while n > 2:
                        k = n // 2
                        nc.vector.tensor_tensor(
                            flat[:, 0:k, :], flat[:, 0:k, :],
                            flat[:, n - k:n, :], op=AL.add)
                        n -= k
                    if first:
                        nc.vector.tensor_tensor(num_acc[:], flat[:, 0, :],
                                                flat[:, n - 1, :], op=AL.add)
                        first = False
                    else:
                        nc.vector.tensor_tensor(flat[:, 0, :], flat[:, 0, :],
                                                flat[:, n - 1, :], op=AL.add)
                        nc.vector.tensor_tensor(num_acc[:], num_acc[:], flat[:, 0, :],
                                                op=AL.add)

                # ---- window close (num cols h*33+f, denom col h*33+32) ----
                nv = num_acc[:].rearrange("p (h f) -> p h f", h=HEADS)
                dn_src = nv[:, :, 32:33].rearrange("p h a -> p (h a)")
                rec = outp.tile([128, HEADS], f32, tag="rec")
                nc.vector.reciprocal(rec[:], dn_src)
                outw = outp.tile([128, 128], f32, tag="outw")
                outw_v = outw[:].rearrange("p (h f) -> p h f", h=HEADS)
                rec_bc = rec[:].rearrange("p (h a) -> p h a", a=1) \
                               .broadcast_to([128, HEADS, OUT_F])
                nc.vector.tensor_tensor(outw_v, nv[:, :, 0:32], rec_bc, op=AL.mult)
                out2 = outp.tile([128, 128], f32, tag="out2")
                nc.vector.tensor_tensor(out2[:], outw[:], res_ps[:], op=AL.add)
                nc.sync.dma_start(out_d.ap()[w * 128:(w + 1) * 128, :], out2[:])

    nc.compile()
    return nc


def kernel(**inputs):
    from concourse.bass_utils import run_bass_kernel_spmd

    args = {k: np.asarray(v) for k, v in inputs.items()}
    common, per_core = _host_preprocess(
        args["x"], args["edge_index"], args["edge_attr"], args["W_lin"],
        args["w_s"], args["b_s"], args["w_t"], args["b_t"], args["W_edge"],
        args["w_e"], args["b_e"], args["W_res"], args["bias"])

    nc = _build_program(common)

    in_maps = []
    for c in range(NCORES):
        pc = per_core[c]
        in_maps.append({
            "xT": pc["xT"], "idx16": pc["idx16"], "l8": pc["l8"], "ea": pc["ea"],
            "wlind": common["wlind"], "dext": common["dext"],
            "crep": common["crep"], "wrese": common["wrese"],
        })

    res = run_bass_kernel_spmd(nc, in_maps, list(range(NCORES)),
                               trace=bool(os.environ.get("GAT_TRACE")),
                               tmpdir=os.environ.get("GAT_TMPDIR"))
    if os.environ.get("GAT_TRACE"):
        print(f"HW exec time: {res.exec_time_ns} ns")

    out = np.empty((N, HEADS * OUT_F), np.float32)
    for c in range(NCORES):
        out[per_core[c]["perm_owned"]] = res.results[c]["out"][:NODES_PC]
    return out


# revision 21
# speedup vs baseline: 1.3849x; 1.0267x over previous
"""GAT layer (gnn_message_passing) on 8 trn2 NeuronCores.

Strategy (dst-sharded, no collectives), v2:
- Each core owns a contiguous 1/8 slice of target nodes; host buckets edges by
  dst core. Within a core, owned nodes are sorted by in-degree (descending) and
  grouped into 128-node windows; node -> SBUF partition, its in-edges occupy
  "slot columns" t=0..deg-1 of that partition.
- Per edge slot, a 768B bf16 row [xp[2j] | xp[2j+1] | a_s[2j] | a_s[2j+1] |
  pad] is fetched with SWDGE dma_gather (idx = perm_pos(src)>>1, int16).
  Gather calls are kept small (<=256 descs) and round-robin over 4 queues to
  stay under the SWDGE ring-stall knee.
- Parity (which half of the pair is the real src) and slot validity are folded
  into an additive logit mask L8 (0 or -100): exp(u - 100) == 0, so no select
  masks and no wasted multiplies.
- Attention logits u = a_s(row) + a_t+biases(pass-0 col) + a_e(edge-attr
  grouped reduce) computed in [128, t, 2, 4] bf16; leaky-relu via STT.
- ACT (Scalar) engine computes exp with a pre-EXPANDED output written straight
  into the rhs buffer in an interleaved [head, 32 feat + 1 denom] = 132-col
  layout; DVE then multiplies rhs by xs in place (STT, 4x bf16 mode) and
  pairwise-folds slots (STT adds). Numerator + denominator come out of one
  fold. Residual x @ W_res.T + bias via ones-row-extended matmul (PE, bf16).
"""
import os
import sys
from contextlib import ExitStack

sys.path.insert(0, "/opt/trn_rl_repo")

import numpy as np

N, E = 50000, 1600000
IN_F, EDGE_F, HEADS, OUT_F = 64, 16, 4, 32
NEG_SLOPE = 0.2
NCORES = 8
NODES_PC = N // NCORES            # 6250
NW = (NODES_PC + 127) // 128      # 49 windows/core
WNODES = NW * 128                 # 6272 (last window partially real)
TC_TILES = 40                     # compute-chunk size in 128-slot tiles
GCALL_TILES = 2                   # tiles per dma_gather call (256 descs)
# bf16 row: [xp_lo(128) | as_lo(4) | xp_hi(128) | as_hi(4) | pad(120)] so each
# parity is one contiguous 132-col block (single pass-0 write per parity).
ROWF = 384
# exp(LMASK) must stay nonzero in bf16 so empty segments keep a nonzero
# denominator (no clamp needed): exp(-80) ~ 1.8e-35 > bf16 min normal.
LMASK = -80.0


def _bf16(a):
    import ml_dtypes
    return np.ascontiguousarray(np.asarray(a, np.float32).astype(ml_dtypes.bfloat16))


def _host_preprocess(x, edge_index, edge_attr, W_lin, w_s, b_s, w_t, b_t,
                     W_edge, w_e, b_e, W_res, bias):
    """Pure index/layout work + weight folding. Returns (common, per_core)."""
    src = edge_index[0].astype(np.int64)
    dst = edge_index[1].astype(np.int64)
    deg = np.bincount(dst, minlength=N)

    # ---- weight folding (weights only; standard operator fusion) ----
    wlinT = np.ascontiguousarray(W_lin.T)                      # [64, 128]
    C = (W_edge.reshape(HEADS, OUT_F, EDGE_F) * w_e[None, :, None]).sum(1)  # [4,16]
    crep = np.tile(C.reshape(-1)[None, :], (128, 1)).astype(np.float32)    # [128,64]
    D = (W_lin.reshape(HEADS, OUT_F, IN_F) * w_t[None, :, None]).sum(1).T  # [64,4]
    b_total = float(b_s) + float(b_t) + float(b_e)
    dext = np.vstack([D, np.full((1, HEADS), b_total, np.float32)]).astype(np.float32)
    Dws = (W_lin.reshape(HEADS, OUT_F, IN_F) * w_s[None, :, None]).sum(1).T  # [64,4]
    wlind = np.concatenate([wlinT.astype(np.float32), Dws.astype(np.float32)],
                           axis=1)                                          # [64,132]
    wrese = np.vstack([W_res.T, bias[None, :]]).astype(np.float32)         # [65,128]

    # ---- per-core schedules (common T_w across cores) ----
    cores = []
    for c in range(NCORES):
        lo = c * NODES_PC
        owned = np.arange(lo, lo + NODES_PC)
        dc = deg[owned]
        order = np.argsort(-dc, kind="stable")
        perm_owned = owned[order]
        degs_sorted = dc[order]
        tw = np.maximum(degs_sorted[::128][:NW], 1).astype(np.int64)
        cores.append(dict(perm_owned=perm_owned, tw=tw))

    T_w = np.max(np.stack([cc["tw"] for cc in cores]), axis=0)  # [NW]
    TOFF = np.concatenate([[0], np.cumsum(T_w)])                # slot col offsets
    SUMT = int(TOFF[-1])

    chunks = []           # (w, t0, t1)
    for w in range(NW):
        t = 0
        while t < T_w[w]:
            t1 = min(t + TC_TILES, int(T_w[w]))
            chunks.append((w, t, t1))
            t = t1

    per_core = []
    for c in range(NCORES):
        cc = cores[c]
        perm_owned = cc["perm_owned"]
        rest = np.setdiff1d(np.arange(N), perm_owned, assume_unique=True)
        perm = np.concatenate([perm_owned, rest])
        perm_pos = np.empty(N, np.int64)
        perm_pos[perm] = np.arange(N)

        emask = (dst >= c * NODES_PC) & (dst < (c + 1) * NODES_PC)
        e_ids = np.nonzero(emask)[0]
        d_loc = perm_pos[dst[e_ids]]                 # 0..6249
        eorder = np.argsort(d_loc, kind="stable")
        e_s = e_ids[eorder]
        ds = d_loc[eorder]
        starts = np.searchsorted(ds, np.arange(NODES_PC))
        t_of = np.arange(len(ds)) - starts[ds]
        w_of = ds // 128
        p_of = ds % 128
        col = TOFF[w_of] + t_of

        src_rel = perm_pos[src[e_s]]
        par = (src_rel & 1).astype(np.int64)

        idx_slot = np.zeros((128, SUMT), np.int16)
        idx_slot[p_of, col] = (src_rel >> 1).astype(np.int16)

        # additive logit mask: [128, SUMT, 2(parity half), 4(heads)]
        l8 = np.full((128, SUMT, 2, HEADS), np.float32(LMASK), np.float32)
        l8[p_of, col, par] = 0.0

        ea_slot = np.zeros((128, SUMT, EDGE_F), np.float32)
        ea_slot[p_of, col] = edge_attr[e_s]

        # idx wrapped in 16 partitions (per window), replicated x8
        idx16 = np.zeros((128, SUMT * 8), np.int16)
        for w in range(NW):
            t0, t1 = int(TOFF[w]), int(TOFF[w + 1])
            flat = idx_slot[:, t0:t1].T.reshape(-1)
            wrapped = flat.reshape(-1, 16).T          # [16, T_w*8]
            idx16[:, t0 * 8: t1 * 8] = np.tile(wrapped, (8, 1))

        xT_ext = np.empty((IN_F + 1, N), np.float32)
        xT_ext[:IN_F] = x[perm].T
        xT_ext[IN_F] = 1.0

        per_core.append(dict(
            xT=_bf16(xT_ext),
            idx16=idx16,
            l8=_bf16(l8.reshape(128, SUMT * 8)),
            ea=_bf16(ea_slot.reshape(128, SUMT * EDGE_F)),
            perm_owned=perm_owned,
        ))

    common = dict(T_w=T_w, TOFF=TOFF, SUMT=SUMT, chunks=chunks,
                  wlind=_bf16(wlind), dext=_bf16(dext), crep=_bf16(crep),
                  wrese=_bf16(wrese))
    return common, per_core


def _build_program(common):
    import concourse.bass as bass
    import concourse.tile as tile
    from concourse import bacc, mybir

    f32 = mybir.dt.float32
    bf16 = mybir.dt.bfloat16
    i16 = mybir.dt.int16
    AL = mybir.AluOpType
    SUMT = common["SUMT"]
    T_w, TOFF, chunks = common["T_w"], common["TOFF"], common["chunks"]

    nc = bacc.Bacc("TRN2", target_bir_lowering=False, debug=False,
                   num_devices=NCORES, num_swdge_queues=4)

    xT_d = nc.dram_tensor("xT", [IN_F + 1, N], bf16, kind="ExternalInput")
    idx_d = nc.dram_tensor("idx16", [128, SUMT * 8], i16, kind="ExternalInput")
    l8_d = nc.dram_tensor("l8", [128, SUMT * 8], bf16, kind="ExternalInput")
    ea_d = nc.dram_tensor("ea", [128, SUMT * EDGE_F], bf16, kind="ExternalInput")
    wlin_d = nc.dram_tensor("wlind", [IN_F, 132], bf16, kind="ExternalInput")
    dext_d = nc.dram_tensor("dext", [IN_F + 1, HEADS], bf16, kind="ExternalInput")
    crep_d = nc.dram_tensor("crep", [128, HEADS * EDGE_F], bf16, kind="ExternalInput")
    wrese_d = nc.dram_tensor("wrese", [IN_F + 1, 128], bf16, kind="ExternalInput")
    out_d = nc.dram_tensor("out", [WNODES, 128], f32, kind="ExternalOutput")

    with tile.TileContext(nc) as tc, ExitStack() as ctx:
        const = ctx.enter_context(tc.tile_pool(name="const", bufs=1))
        dramp = ctx.enter_context(tc.tile_pool(name="dram", bufs=1, space="DRAM"))
        xp_t = dramp.tile([N // 2, ROWF], bf16)

        wlint = const.tile([IN_F, 132], bf16)
        nc.sync.dma_start(wlint[:], wlin_d.ap())
        dext_t = const.tile([IN_F + 1, HEADS], bf16)
        nc.sync.dma_start(dext_t[:], dext_d.ap())
        crep_t = const.tile([128, HEADS * EDGE_F], bf16)
        nc.sync.dma_start(crep_t[:], crep_d.ap())
        wrese_t = const.tile([IN_F + 1, 128], bf16)
        nc.sync.dma_start(wrese_t[:], wrese_d.ap())
        xTown = const.tile([IN_F + 1, WNODES], bf16)
        nc.sync.dma_start(xTown[:], xT_d.ap()[:, 0:WNODES])
        l8t = const.tile([128, SUMT * 8], bf16)
        nc.sync.dma_start(l8t[:], l8_d.ap())
        atb = const.tile([128, NW * HEADS], bf16)

        # ---- pass-0: gather table ([25000, 384] bf16 pair rows) + a_t ----
        NBLK = (N + 127) // 128          # 391 node blocks of 128
        GB = 16                          # blocks per batched table write
        SLABW = 12544                    # 98 blocks per slab
        with tc.tile_pool(name="p0slab", bufs=2) as slabp, \
             tc.tile_pool(name="p0", bufs=3) as p0, \
             tc.tile_pool(name="p0ps", bufs=4, space="PSUM") as p0ps:
            xp_flat = xp_t[:]            # [25000, 384]
            nslab = (N + SLABW - 1) // SLABW
            for sl in range(nslab):
                c0 = sl * SLABW
                cw = min(SLABW, N - c0)
                slab = slabp.tile([IN_F, SLABW], bf16, tag="slab")
                nc.sync.dma_start(slab[:, :cw], xT_d.ap()[0:IN_F, c0:c0 + cw])
                b0 = c0 // 128
                bn = (cw + 127) // 128
                for bg in range(b0, b0 + bn, GB):
                    gn = min(GB, b0 + bn - bg)
                    stage = p0.tile([128, GB * 132], bf16, tag="stage")
                    for k in range(gn):
                        b = bg + k
                        nb = min(128, N - b * 128)
                        lo = b * 128 - c0
                        if nb < 128:
                            nc.vector.memset(stage[:, k * 132:(k + 1) * 132], 0.0)
                        ps = p0ps.tile([128, 132], f32, tag="ps")
                        nc.tensor.matmul(ps[:nb, :], slab[:, lo:lo + nb],
                                         wlint[:], start=True, stop=True)
                        if k % 2 == 0:
                            nc.scalar.copy(stage[:nb, k * 132:(k + 1) * 132], ps[:nb, :])
                        else:
                            nc.vector.tensor_copy(stage[:nb, k * 132:(k + 1) * 132], ps[:nb, :])
                    gfull = gn
                    if bg + gn == NBLK and N % 128 != 0:
                        gfull = gn - 1
                    for par in range(2):
                        src = stage[:].rearrange("(r a) c -> a r c", a=2)[par] \
                                      .rearrange("r (k c) -> r k c", c=132)
                        if gfull > 0:
                            dst_xp = xp_flat[64 * bg: 64 * (bg + gfull),
                                             132 * par: 132 * par + 132] \
                                .rearrange("(k r) f -> r k f", k=gfull)
                            nc.sync.dma_start(dst_xp, src[:, :gfull, 0:132])
                        if gfull < gn:
                            b = bg + gfull
                            rows = (N - b * 128) // 2     # pair rows in partial block
                            r0 = 64 * b
                            nc.sync.dma_start(
                                xp_flat[r0: r0 + rows, 132 * par: 132 * par + 132],
                                src[:rows, gfull, 0:132])
            for w in range(NW):
                ps2 = p0ps.tile([128, HEADS], f32, tag="ps2")
                nc.tensor.matmul(ps2[:], xTown[:, w * 128:(w + 1) * 128], dext_t[:],
                                 start=True, stop=True)
                nc.scalar.copy(atb[:, w * HEADS:(w + 1) * HEADS], ps2[:])

        # ---- main loop ----
        with tc.tile_pool(name="xsp", bufs=3) as xsp, \
             tc.tile_pool(name="eap", bufs=4) as eap, \
             tc.tile_pool(name="idxp", bufs=4) as idxp, \
             tc.tile_pool(name="scr", bufs=2) as scr, \
             tc.tile_pool(name="sml", bufs=3) as sml, \
             tc.tile_pool(name="rhsp", bufs=2) as rhsp, \
             tc.tile_pool(name="nap", bufs=3) as nap, \
             tc.tile_pool(name="outp", bufs=4) as outp, \
             tc.tile_pool(name="mps", bufs=3, space="PSUM") as mps:

            # Software pipeline: phase A(c) = gathers + logits + ACT exp for
            # chunk c; phase B(c) = msg multiply + fold + window close. A(c+1)
            # is emitted before B(c) so each in-order engine queue interleaves
            # work of two chunks: DVE never parks at msg(c) waiting on ACT's
            # exp(c) with ready work behind it, and ACT's exp(c+1) overlaps
            # DVE's fold(c).
            qst = [0]
            CH = list(chunks)
            state = [None] * len(CH)
            win_res = {}
            win_num = {}

            def phase_a(ci):
                w, t0, t1 = CH[ci]
                tcn = t1 - t0
                scol = int(TOFF[w]) + t0
                icol = scol * 8
                if t0 == 0:
                    res_ps = mps.tile([128, 128], f32, tag="res")
                    nc.tensor.matmul(res_ps[:], xTown[:, w * 128:(w + 1) * 128],
                                     wrese_t[:], start=True, stop=True)
                    win_res[w] = res_ps

                idxc = idxp.tile([128, TC_TILES * 8], i16, tag="idxc")
                nc.sync.dma_start(idxc[:, :tcn * 8],
                                  idx_d.ap()[:, icol: icol + tcn * 8])
                xs = xsp.tile([128, TC_TILES, ROWF], bf16, tag="xs")
                tpos = 0
                while tpos < tcn:
                    tn = min(GCALL_TILES, tcn - tpos)
                    nc.gpsimd.dma_gather(
                        xs[:, tpos:tpos + tn, :], xp_t[:],
                        idxc[:, tpos * 8:(tpos + tn) * 8],
                        tn * 128, tn * 128, ROWF, single_packet=True,
                        queue_num=qst[0] % 4)
                    qst[0] += 1
                    tpos += tn

                eat = eap.tile([128, TC_TILES * EDGE_F], bf16, tag="eat")
                nc.sync.dma_start(eat[:, :tcn * EDGE_F],
                                  ea_d.ap()[:, scol * EDGE_F: (scol + tcn) * EDGE_F])

                # a_e: grouped product + tree reduce (TT, bf16 2x)
                prode = scr.tile([128, TC_TILES * HEADS * EDGE_F], bf16, tag="prode")
                ea_bc = eat[:, :tcn * EDGE_F] \
                    .rearrange("p (t k) -> p t k", t=tcn) \
                    .rearrange("p t (a k) -> p t a k", a=1) \
                    .broadcast_to([128, tcn, HEADS, EDGE_F])
                crep_bc = crep_t[:].rearrange("p (a f) -> p a f", a=1) \
                    .broadcast_to([128, tcn, HEADS * EDGE_F]) \
                    .rearrange("p t (h k) -> p t h k", h=HEADS)
                prode_v = prode[:, :tcn * HEADS * EDGE_F] \
                    .rearrange("p (t h k) -> p t h k", t=tcn, h=HEADS)
                nc.vector.tensor_tensor(prode_v, ea_bc, crep_bc, op=AL.mult)
                kk = EDGE_F
                while kk > 1:
                    half = kk // 2
                    nc.vector.tensor_tensor(
                        prode_v[:, :, :, 0:half], prode_v[:, :, :, 0:half],
                        prode_v[:, :, :, kk - half:kk], op=AL.add)
                    kk -= half

                # u8 = a_s(row) + ze + atb + L8 ; lrelu; [128, t, 2, 4]
                ze_b = prode_v[:, :, :, 0:1] \
                    .rearrange("p t h a -> p t (h a)") \
                    .rearrange("p t (a h) -> p t a h", a=1) \
                    .broadcast_to([128, tcn, 2, HEADS])
                atb_b = atb[:, w * HEADS:(w + 1) * HEADS] \
                    .rearrange("p (a b h) -> p a b h", a=1, b=1) \
                    .broadcast_to([128, tcn, 2, HEADS])
                l8_b = l8t[:, scol * 8:(scol + tcn) * 8] \
                    .rearrange("p (t a h) -> p t a h", t=tcn, a=2)
                xs264 = xs[:, :tcn, 0:264].rearrange("p t (a q) -> p t a q", a=2)
                as8 = xs264[:, :, :, 128:132]
                u8 = sml.tile([128, TC_TILES * 8], bf16, tag="u8")
                u8_v = u8[:, :tcn * 8].rearrange("p (t a h) -> p t a h", t=tcn, a=2)
                nc.vector.tensor_tensor(u8_v, l8_b, atb_b, op=AL.add)
                nc.vector.tensor_tensor(u8_v, u8_v, ze_b, op=AL.add)
                nc.vector.tensor_tensor(u8_v, u8_v, as8, op=AL.add)
                u8_f = u8[:, :tcn * 8]
                nc.vector.scalar_tensor_tensor(u8_f, u8_f, NEG_SLOPE, u8_f,
                                               op0=AL.mult, op1=AL.max)

                # exp with expanded output -> rhs[p, t, 2, 4, 33]
                rhs = rhsp.tile([128, TC_TILES, 2, 132], bf16, tag="rhs")
                rhs_e = rhs[:, :tcn, :, :].rearrange("p t a (h f) -> p t a h f", h=HEADS)
                u8_bc = u8_v.rearrange("p t a (h f) -> p t a h f", f=1) \
                    .broadcast_to([128, tcn, 2, HEADS, 33])
                nc.scalar.activation(rhs_e, u8_bc,
                                     mybir.ActivationFunctionType.Exp)
                state[ci] = (w, t0, t1, tcn, xs264, rhs, rhs_e)

            def phase_b(ci):
                w, t0, t1, tcn, xs264, rhs, rhs_e = state[ci]
                state[ci] = None
                # msg: rhs[..., h, 0:32] *= xs pair halves (TT, bf16 2x)
                msg_v = rhs_e[:, :, :, :, 0:32]
                xs_v = xs264[:, :, :, 0:128] \
                    .rearrange("p t a (h f) -> p t a h f", h=HEADS)
                nc.vector.tensor_tensor(msg_v, msg_v, xs_v, op=AL.mult)

                # fold slots: [128, 2t, 132] -> num_acc (TT adds, bf16 2x)
                flat = rhs[:, :tcn, :, :].rearrange("p t h f -> p (t h) f")
                n = 2 * tcn
                while n > 2:
                    k = n // 2
                    nc.vector.tensor_tensor(
                        flat[:, 0:k, :], flat[:, 0:k, :],
                        flat[:, n - k:n, :], op=AL.add)
                    n -= k
                if t0 == 0:
                    num_acc = nap.tile([128, 132], f32, tag="num")
                    win_num[w] = num_acc
                    nc.vector.tensor_tensor(num_acc[:], flat[:, 0, :],
                                            flat[:, n - 1, :], op=AL.add)
                else:
                    num_acc = win_num[w]
                    nc.vector.tensor_tensor(flat[:, 0, :], flat[:, 0, :],
                                            flat[:, n - 1, :], op=AL.add)
                    nc.vector.tensor_tensor(num_acc[:], num_acc[:], flat[:, 0, :],
                                            op=AL.add)
                if t1 != int(T_w[w]):
                    return
                # ---- window close (num cols h*33+f, denom col h*33+32) ----
                nv = num_acc[:].rearrange("p (h f) -> p h f", h=HEADS)
                dn_src = nv[:, :, 32:33].rearrange("p h a -> p (h a)")
                rec = outp.tile([128, HEADS], f32, tag="rec")
                nc.vector.reciprocal(rec[:], dn_src)
                outw = outp.tile([128, 128], f32, tag="outw")
                outw_v = outw[:].rearrange("p (h f) -> p h f", h=HEADS)
                rec_bc = rec[:].rearrange("p (h a) -> p h a", a=1) \
                               .broadcast_to([128, HEADS, OUT_F])
                nc.vector.tensor_tensor(outw_v, nv[:, :, 0:32], rec_bc, op=AL.mult)
                out2 = outp.tile([128, 128], f32, tag="out2")
                nc.vector.tensor_tensor(out2[:], outw[:], win_res.pop(w)[:], op=AL.add)
                nc.sync.dma_start(out_d.ap()[w * 128:(w + 1) * 128, :], out2[:])

            phase_a(0)
            for ci in range(len(CH)):
                if ci + 1 < len(CH):
                    phase_a(ci + 1)
                phase_b(ci)

    nc.compile()
    return nc


def kernel(**inputs):
    from concourse.bass_utils import run_bass_kernel_spmd

    args = {k: np.asarray(v) for k, v in inputs.items()}
    common, per_core = _host_preprocess(
        args["x"], args["edge_index"], args["edge_attr"], args["W_lin"],
        args["w_s"], args["b_s"], args["w_t"], args["b_t"], args["W_edge"],
        args["w_e"], args["b_e"], args["W_res"], args["bias"])

    nc = _build_program(common)

    in_maps = []
    for c in range(NCORES):
        pc = per_core[c]
        in_maps.append({
            "xT": pc["xT"], "idx16": pc["idx16"], "l8": pc["l8"], "ea": pc["ea"],
            "wlind": common["wlind"], "dext": common["dext"],
            "crep": common["crep"], "wrese": common["wrese"],
        })

    res = run_bass_kernel_spmd(nc, in_maps, list(range(NCORES)),
                               trace=bool(os.environ.get("GAT_TRACE")),
                               tmpdir=os.environ.get("GAT_TMPDIR"))
    if os.environ.get("GAT_TRACE"):
        print(f"HW exec time: {res.exec_time_ns} ns")

    out = np.empty((N, HEADS * OUT_F), np.float32)
    for c in range(NCORES):
        out[per_core[c]["perm_owned"]] = res.results[c]["out"][:NODES_PC]
    return out


# revision 22
# speedup vs baseline: 1.4848x; 1.0721x over previous
"""GAT layer (gnn_message_passing) on 8 trn2 NeuronCores.

Strategy (dst-sharded, no collectives), v2:
- Each core owns a contiguous 1/8 slice of target nodes; host buckets edges by
  dst core. Within a core, owned nodes are sorted by in-degree (descending) and
  grouped into 128-node windows; node -> SBUF partition, its in-edges occupy
  "slot columns" t=0..deg-1 of that partition.
- Per edge slot, a 768B bf16 row [xp[2j] | xp[2j+1] | a_s[2j] | a_s[2j+1] |
  pad] is fetched with SWDGE dma_gather (idx = perm_pos(src)>>1, int16).
  Gather calls are kept small (<=256 descs) and round-robin over 4 queues to
  stay under the SWDGE ring-stall knee.
- Parity (which half of the pair is the real src) and slot validity are folded
  into an additive logit mask L8 (0 or -100): exp(u - 100) == 0, so no select
  masks and no wasted multiplies.
- Attention logits u = a_s(row) + a_t+biases(pass-0 col) + a_e(edge-attr
  grouped reduce) computed in [128, t, 2, 4] bf16; leaky-relu via STT.
- ACT (Scalar) engine computes exp with a pre-EXPANDED output written straight
  into the rhs buffer in an interleaved [head, 32 feat + 1 denom] = 132-col
  layout; DVE then multiplies rhs by xs in place (STT, 4x bf16 mode) and
  pairwise-folds slots (STT adds). Numerator + denominator come out of one
  fold. Residual x @ W_res.T + bias via ones-row-extended matmul (PE, bf16).
"""
import os
import sys
from contextlib import ExitStack

sys.path.insert(0, "/opt/trn_rl_repo")

import numpy as np

N, E = 50000, 1600000
IN_F, EDGE_F, HEADS, OUT_F = 64, 16, 4, 32
NEG_SLOPE = 0.2
NCORES = 8
NODES_PC = N // NCORES            # 6250
NW = (NODES_PC + 127) // 128      # 49 windows/core
WNODES = NW * 128                 # 6272 (last window partially real)
TC_TILES = 32                     # compute-chunk size in 128-slot tiles
GCALL_TILES = 2                   # tiles per dma_gather call (256 descs)
# bf16 row: [xp_lo(128) | as_lo(4) | xp_hi(128) | as_hi(4) | pad(120)] so each
# parity is one contiguous 132-col block (single pass-0 write per parity).
ROWF = 384
# exp(LMASK) must stay nonzero in bf16 so empty segments keep a nonzero
# denominator (no clamp needed): exp(-80) ~ 1.8e-35 > bf16 min normal.
LMASK = -80.0


def _bf16(a):
    import ml_dtypes
    return np.ascontiguousarray(np.asarray(a, np.float32).astype(ml_dtypes.bfloat16))


def _host_preprocess(x, edge_index, edge_attr, W_lin, w_s, b_s, w_t, b_t,
                     W_edge, w_e, b_e, W_res, bias):
    """Pure index/layout work + weight folding. Returns (common, per_core)."""
    src = edge_index[0].astype(np.int64)
    dst = edge_index[1].astype(np.int64)
    deg = np.bincount(dst, minlength=N)

    # ---- weight folding (weights only; standard operator fusion) ----
    wlinT = np.ascontiguousarray(W_lin.T)                      # [64, 128]
    C = (W_edge.reshape(HEADS, OUT_F, EDGE_F) * w_e[None, :, None]).sum(1)  # [4,16]
    crep = np.tile(C.reshape(-1)[None, :], (128, 1)).astype(np.float32)    # [128,64]
    D = (W_lin.reshape(HEADS, OUT_F, IN_F) * w_t[None, :, None]).sum(1).T  # [64,4]
    b_total = float(b_s) + float(b_t) + float(b_e)
    dext = np.vstack([D, np.full((1, HEADS), b_total, np.float32)]).astype(np.float32)
    Dws = (W_lin.reshape(HEADS, OUT_F, IN_F) * w_s[None, :, None]).sum(1).T  # [64,4]
    wlind = np.concatenate([wlinT.astype(np.float32), Dws.astype(np.float32)],
                           axis=1)                                          # [64,132]
    wrese = np.vstack([W_res.T, bias[None, :]]).astype(np.float32)         # [65,128]

    # ---- per-core schedules (common T_w across cores) ----
    cores = []
    for c in range(NCORES):
        lo = c * NODES_PC
        owned = np.arange(lo, lo + NODES_PC)
        dc = deg[owned]
        order = np.argsort(-dc, kind="stable")
        perm_owned = owned[order]
        degs_sorted = dc[order]
        tw = np.maximum(degs_sorted[::128][:NW], 1).astype(np.int64)
        cores.append(dict(perm_owned=perm_owned, tw=tw))

    T_w = np.max(np.stack([cc["tw"] for cc in cores]), axis=0)  # [NW]
    TOFF = np.concatenate([[0], np.cumsum(T_w)])                # slot col offsets
    SUMT = int(TOFF[-1])

    chunks = []           # (w, t0, t1)
    for w in range(NW):
        t = 0
        while t < T_w[w]:
            t1 = min(t + TC_TILES, int(T_w[w]))
            chunks.append((w, t, t1))
            t = t1

    per_core = []
    for c in range(NCORES):
        cc = cores[c]
        perm_owned = cc["perm_owned"]
        rest = np.setdiff1d(np.arange(N), perm_owned, assume_unique=True)
        perm = np.concatenate([perm_owned, rest])
        perm_pos = np.empty(N, np.int64)
        perm_pos[perm] = np.arange(N)

        emask = (dst >= c * NODES_PC) & (dst < (c + 1) * NODES_PC)
        e_ids = np.nonzero(emask)[0]
        d_loc = perm_pos[dst[e_ids]]                 # 0..6249
        eorder = np.argsort(d_loc, kind="stable")
        e_s = e_ids[eorder]
        ds = d_loc[eorder]
        starts = np.searchsorted(ds, np.arange(NODES_PC))
        t_of = np.arange(len(ds)) - starts[ds]
        w_of = ds // 128
        p_of = ds % 128
        col = TOFF[w_of] + t_of

        src_rel = perm_pos[src[e_s]]
        par = (src_rel & 1).astype(np.int64)

        idx_slot = np.zeros((128, SUMT), np.int16)
        idx_slot[p_of, col] = (src_rel >> 1).astype(np.int16)

        # additive logit mask: [128, SUMT, 2(parity half), 4(heads)]
        l8 = np.full((128, SUMT, 2, HEADS), np.float32(LMASK), np.float32)
        l8[p_of, col, par] = 0.0

        ea_slot = np.zeros((128, SUMT, EDGE_F), np.float32)
        ea_slot[p_of, col] = edge_attr[e_s]

        # idx wrapped in 16 partitions (per window), replicated x8
        idx16 = np.zeros((128, SUMT * 8), np.int16)
        for w in range(NW):
            t0, t1 = int(TOFF[w]), int(TOFF[w + 1])
            flat = idx_slot[:, t0:t1].T.reshape(-1)
            wrapped = flat.reshape(-1, 16).T          # [16, T_w*8]
            idx16[:, t0 * 8: t1 * 8] = np.tile(wrapped, (8, 1))

        xT_ext = np.empty((IN_F + 1, N), np.float32)
        xT_ext[:IN_F] = x[perm].T
        xT_ext[IN_F] = 1.0

        per_core.append(dict(
            xT=_bf16(xT_ext),
            idx16=idx16,
            l8=_bf16(l8.reshape(128, SUMT * 8)),
            ea=_bf16(ea_slot.reshape(128, SUMT * EDGE_F)),
            perm_owned=perm_owned,
        ))

    common = dict(T_w=T_w, TOFF=TOFF, SUMT=SUMT, chunks=chunks,
                  wlind=_bf16(wlind), dext=_bf16(dext), crep=_bf16(crep),
                  wrese=_bf16(wrese))
    return common, per_core


def _build_program(common):
    import concourse.bass as bass
    import concourse.tile as tile
    from concourse import bacc, mybir

    f32 = mybir.dt.float32
    bf16 = mybir.dt.bfloat16
    i16 = mybir.dt.int16
    AL = mybir.AluOpType
    SUMT = common["SUMT"]
    T_w, TOFF, chunks = common["T_w"], common["TOFF"], common["chunks"]

    nc = bacc.Bacc("TRN2", target_bir_lowering=False, debug=False,
                   num_devices=NCORES, num_swdge_queues=4)

    xT_d = nc.dram_tensor("xT", [IN_F + 1, N], bf16, kind="ExternalInput")
    idx_d = nc.dram_tensor("idx16", [128, SUMT * 8], i16, kind="ExternalInput")
    l8_d = nc.dram_tensor("l8", [128, SUMT * 8], bf16, kind="ExternalInput")
    ea_d = nc.dram_tensor("ea", [128, SUMT * EDGE_F], bf16, kind="ExternalInput")
    wlin_d = nc.dram_tensor("wlind", [IN_F, 132], bf16, kind="ExternalInput")
    dext_d = nc.dram_tensor("dext", [IN_F + 1, HEADS], bf16, kind="ExternalInput")
    crep_d = nc.dram_tensor("crep", [128, HEADS * EDGE_F], bf16, kind="ExternalInput")
    wrese_d = nc.dram_tensor("wrese", [IN_F + 1, 128], bf16, kind="ExternalInput")
    out_d = nc.dram_tensor("out", [WNODES, 128], f32, kind="ExternalOutput")

    with tile.TileContext(nc) as tc, ExitStack() as ctx:
        const = ctx.enter_context(tc.tile_pool(name="const", bufs=1))
        dramp = ctx.enter_context(tc.tile_pool(name="dram", bufs=1, space="DRAM"))
        xp_t = dramp.tile([N // 2, ROWF], bf16)

        wlint = const.tile([IN_F, 132], bf16)
        nc.sync.dma_start(wlint[:], wlin_d.ap())
        dext_t = const.tile([IN_F + 1, HEADS], bf16)
        nc.sync.dma_start(dext_t[:], dext_d.ap())
        crep_t = const.tile([128, HEADS * EDGE_F], bf16)
        nc.sync.dma_start(crep_t[:], crep_d.ap())
        wrese_t = const.tile([IN_F + 1, 128], bf16)
        nc.sync.dma_start(wrese_t[:], wrese_d.ap())
        xTown = const.tile([IN_F + 1, WNODES], bf16)
        nc.sync.dma_start(xTown[:], xT_d.ap()[:, 0:WNODES])
        l8t = const.tile([128, SUMT * 8], bf16)
        nc.sync.dma_start(l8t[:], l8_d.ap())
        atb = const.tile([128, NW * HEADS], bf16)

        # ---- pass-0: gather table ([25000, 384] bf16 pair rows) + a_t ----
        NBLK = (N + 127) // 128          # 391 node blocks of 128
        GB = 16                          # blocks per batched table write
        SLABW = 12544                    # 98 blocks per slab
        with tc.tile_pool(name="p0slab", bufs=2) as slabp, \
             tc.tile_pool(name="p0", bufs=3) as p0, \
             tc.tile_pool(name="p0ps", bufs=4, space="PSUM") as p0ps:
            xp_flat = xp_t[:]            # [25000, 384]
            nslab = (N + SLABW - 1) // SLABW
            for sl in range(nslab):
                c0 = sl * SLABW
                cw = min(SLABW, N - c0)
                slab = slabp.tile([IN_F, SLABW], bf16, tag="slab")
                nc.sync.dma_start(slab[:, :cw], xT_d.ap()[0:IN_F, c0:c0 + cw])
                b0 = c0 // 128
                bn = (cw + 127) // 128
                for bg in range(b0, b0 + bn, GB):
                    gn = min(GB, b0 + bn - bg)
                    stage = p0.tile([128, GB * 132], bf16, tag="stage")
                    for k in range(gn):
                        b = bg + k
                        nb = min(128, N - b * 128)
                        lo = b * 128 - c0
                        if nb < 128:
                            nc.vector.memset(stage[:, k * 132:(k + 1) * 132], 0.0)
                        ps = p0ps.tile([128, 132], f32, tag="ps")
                        nc.tensor.matmul(ps[:nb, :], slab[:, lo:lo + nb],
                                         wlint[:], start=True, stop=True)
                        if k % 2 == 0:
                            nc.scalar.copy(stage[:nb, k * 132:(k + 1) * 132], ps[:nb, :])
                        else:
                            nc.vector.tensor_copy(stage[:nb, k * 132:(k + 1) * 132], ps[:nb, :])
                    gfull = gn
                    if bg + gn == NBLK and N % 128 != 0:
                        gfull = gn - 1
                    for par in range(2):
                        src = stage[:].rearrange("(r a) c -> a r c", a=2)[par] \
                                      .rearrange("r (k c) -> r k c", c=132)
                        if gfull > 0:
                            dst_xp = xp_flat[64 * bg: 64 * (bg + gfull),
                                             132 * par: 132 * par + 132] \
                                .rearrange("(k r) f -> r k f", k=gfull)
                            nc.sync.dma_start(dst_xp, src[:, :gfull, 0:132])
                        if gfull < gn:
                            b = bg + gfull
                            rows = (N - b * 128) // 2     # pair rows in partial block
                            r0 = 64 * b
                            nc.sync.dma_start(
                                xp_flat[r0: r0 + rows, 132 * par: 132 * par + 132],
                                src[:rows, gfull, 0:132])
            for w in range(NW):
                ps2 = p0ps.tile([128, HEADS], f32, tag="ps2")
                nc.tensor.matmul(ps2[:], xTown[:, w * 128:(w + 1) * 128], dext_t[:],
                                 start=True, stop=True)
                nc.scalar.copy(atb[:, w * HEADS:(w + 1) * HEADS], ps2[:])

        # ---- main loop ----
        with tc.tile_pool(name="xsp", bufs=3) as xsp, \
             tc.tile_pool(name="eap", bufs=4) as eap, \
             tc.tile_pool(name="idxp", bufs=4) as idxp, \
             tc.tile_pool(name="scr", bufs=2) as scr, \
             tc.tile_pool(name="sml", bufs=3) as sml, \
             tc.tile_pool(name="rhsp", bufs=3) as rhsp, \
             tc.tile_pool(name="nap", bufs=3) as nap, \
             tc.tile_pool(name="outp", bufs=4) as outp, \
             tc.tile_pool(name="mps", bufs=3, space="PSUM") as mps:

            # Software pipeline: phase A(c) = gathers + logits + ACT exp for
            # chunk c; phase B(c) = msg multiply + fold + window close. A(c+1)
            # is emitted before B(c) so each in-order engine queue interleaves
            # work of two chunks: DVE never parks at msg(c) waiting on ACT's
            # exp(c) with ready work behind it, and ACT's exp(c+1) overlaps
            # DVE's fold(c).
            qst = [0]
            CH = list(chunks)
            state = [None] * len(CH)
            win_res = {}
            win_num = {}

            def phase_a(ci):
                w, t0, t1 = CH[ci]
                tcn = t1 - t0
                scol = int(TOFF[w]) + t0
                icol = scol * 8
                if t0 == 0:
                    res_ps = mps.tile([128, 128], f32, tag="res")
                    nc.tensor.matmul(res_ps[:], xTown[:, w * 128:(w + 1) * 128],
                                     wrese_t[:], start=True, stop=True)
                    win_res[w] = res_ps

                idxc = idxp.tile([128, TC_TILES * 8], i16, tag="idxc")
                nc.sync.dma_start(idxc[:, :tcn * 8],
                                  idx_d.ap()[:, icol: icol + tcn * 8])
                xs = xsp.tile([128, TC_TILES, ROWF], bf16, tag="xs")
                tpos = 0
                while tpos < tcn:
                    tn = min(GCALL_TILES, tcn - tpos)
                    nc.gpsimd.dma_gather(
                        xs[:, tpos:tpos + tn, :], xp_t[:],
                        idxc[:, tpos * 8:(tpos + tn) * 8],
                        tn * 128, tn * 128, ROWF, single_packet=True,
                        queue_num=qst[0] % 4)
                    qst[0] += 1
                    tpos += tn

                eat = eap.tile([128, TC_TILES * EDGE_F], bf16, tag="eat")
                nc.sync.dma_start(eat[:, :tcn * EDGE_F],
                                  ea_d.ap()[:, scol * EDGE_F: (scol + tcn) * EDGE_F])

                # a_e: grouped product + tree reduce (TT, bf16 2x)
                prode = scr.tile([128, TC_TILES * HEADS * EDGE_F], bf16, tag="prode")
                ea_bc = eat[:, :tcn * EDGE_F] \
                    .rearrange("p (t k) -> p t k", t=tcn) \
                    .rearrange("p t (a k) -> p t a k", a=1) \
                    .broadcast_to([128, tcn, HEADS, EDGE_F])
                crep_bc = crep_t[:].rearrange("p (a f) -> p a f", a=1) \
                    .broadcast_to([128, tcn, HEADS * EDGE_F]) \
                    .rearrange("p t (h k) -> p t h k", h=HEADS)
                prode_v = prode[:, :tcn * HEADS * EDGE_F] \
                    .rearrange("p (t h k) -> p t h k", t=tcn, h=HEADS)
                nc.vector.tensor_tensor(prode_v, ea_bc, crep_bc, op=AL.mult)
                kk = EDGE_F
                while kk > 1:
                    half = kk // 2
                    nc.vector.tensor_tensor(
                        prode_v[:, :, :, 0:half], prode_v[:, :, :, 0:half],
                        prode_v[:, :, :, kk - half:kk], op=AL.add)
                    kk -= half

                # u8 = a_s(row) + ze + atb + L8 ; lrelu; [128, t, 2, 4]
                ze_b = prode_v[:, :, :, 0:1] \
                    .rearrange("p t h a -> p t (h a)") \
                    .rearrange("p t (a h) -> p t a h", a=1) \
                    .broadcast_to([128, tcn, 2, HEADS])
                atb_b = atb[:, w * HEADS:(w + 1) * HEADS] \
                    .rearrange("p (a b h) -> p a b h", a=1, b=1) \
                    .broadcast_to([128, tcn, 2, HEADS])
                l8_b = l8t[:, scol * 8:(scol + tcn) * 8] \
                    .rearrange("p (t a h) -> p t a h", t=tcn, a=2)
                xs264 = xs[:, :tcn, 0:264].rearrange("p t (a q) -> p t a q", a=2)
                as8 = xs264[:, :, :, 128:132]
                u8 = sml.tile([128, TC_TILES * 8], bf16, tag="u8")
                u8_v = u8[:, :tcn * 8].rearrange("p (t a h) -> p t a h", t=tcn, a=2)
                nc.vector.tensor_tensor(u8_v, l8_b, atb_b, op=AL.add)
                nc.vector.tensor_tensor(u8_v, u8_v, ze_b, op=AL.add)
                nc.vector.tensor_tensor(u8_v, u8_v, as8, op=AL.add)
                u8_f = u8[:, :tcn * 8]
                nc.vector.scalar_tensor_tensor(u8_f, u8_f, NEG_SLOPE, u8_f,
                                               op0=AL.mult, op1=AL.max)

                # exp with expanded output -> rhs[p, t, 2, 4, 33]
                rhs = rhsp.tile([128, TC_TILES, 2, 132], bf16, tag="rhs")
                rhs_e = rhs[:, :tcn, :, :].rearrange("p t a (h f) -> p t a h f", h=HEADS)
                u8_bc = u8_v.rearrange("p t a (h f) -> p t a h f", f=1) \
                    .broadcast_to([128, tcn, 2, HEADS, 33])
                nc.scalar.activation(rhs_e, u8_bc,
                                     mybir.ActivationFunctionType.Exp)
                state[ci] = (w, t0, t1, tcn, xs264, rhs, rhs_e)

            def phase_b(ci):
                w, t0, t1, tcn, xs264, rhs, rhs_e = state[ci]
                state[ci] = None
                # msg: rhs[..., h, 0:32] *= xs pair halves (TT, bf16 2x)
                msg_v = rhs_e[:, :, :, :, 0:32]
                xs_v = xs264[:, :, :, 0:128] \
                    .rearrange("p t a (h f) -> p t a h f", h=HEADS)
                nc.vector.tensor_tensor(msg_v, msg_v, xs_v, op=AL.mult)

                # fold slots: [128, 2t, 132] -> num_acc (TT adds, bf16 2x)
                flat = rhs[:, :tcn, :, :].rearrange("p t h f -> p (t h) f")
                n = 2 * tcn
                while n > 2:
                    k = n // 2
                    nc.vector.tensor_tensor(
                        flat[:, 0:k, :], flat[:, 0:k, :],
                        flat[:, n - k:n, :], op=AL.add)
                    n -= k
                if t0 == 0:
                    num_acc = nap.tile([128, 132], f32, tag="num")
                    win_num[w] = num_acc
                    nc.vector.tensor_tensor(num_acc[:], flat[:, 0, :],
                                            flat[:, n - 1, :], op=AL.add)
                else:
                    num_acc = win_num[w]
                    nc.vector.tensor_tensor(flat[:, 0, :], flat[:, 0, :],
                                            flat[:, n - 1, :], op=AL.add)
                    nc.vector.tensor_tensor(num_acc[:], num_acc[:], flat[:, 0, :],
                                            op=AL.add)
                if t1 != int(T_w[w]):
                    return
                # ---- window close (num cols h*33+f, denom col h*33+32) ----
                nv = num_acc[:].rearrange("p (h f) -> p h f", h=HEADS)
                dn_src = nv[:, :, 32:33].rearrange("p h a -> p (h a)")
                rec = outp.tile([128, HEADS], f32, tag="rec")
                nc.vector.reciprocal(rec[:], dn_src)
                outw = outp.tile([128, 128], f32, tag="outw")
                outw_v = outw[:].rearrange("p (h f) -> p h f", h=HEADS)
                rec_bc = rec[:].rearrange("p (h a) -> p h a", a=1) \
                               .broadcast_to([128, HEADS, OUT_F])
                nc.vector.tensor_tensor(outw_v, nv[:, :, 0:32], rec_bc, op=AL.mult)
                out2 = outp.tile([128, 128], f32, tag="out2")
                nc.vector.tensor_tensor(out2[:], outw[:], win_res.pop(w)[:], op=AL.add)
                nc.sync.dma_start(out_d.ap()[w * 128:(w + 1) * 128, :], out2[:])

            phase_a(0)
            for ci in range(len(CH)):
                if ci + 1 < len(CH):
                    phase_a(ci + 1)
                phase_b(ci)

    nc.compile()
    return nc


def kernel(**inputs):
    from concourse.bass_utils import run_bass_kernel_spmd

    args = {k: np.asarray(v) for k, v in inputs.items()}
    common, per_core = _host_preprocess(
        args["x"], args["edge_index"], args["edge_attr"], args["W_lin"],
        args["w_s"], args["b_s"], args["w_t"], args["b_t"], args["W_edge"],
        args["w_e"], args["b_e"], args["W_res"], args["bias"])

    nc = _build_program(common)

    in_maps = []
    for c in range(NCORES):
        pc = per_core[c]
        in_maps.append({
            "xT": pc["xT"], "idx16": pc["idx16"], "l8": pc["l8"], "ea": pc["ea"],
            "wlind": common["wlind"], "dext": common["dext"],
            "crep": common["crep"], "wrese": common["wrese"],
        })

    res = run_bass_kernel_spmd(nc, in_maps, list(range(NCORES)),
                               trace=bool(os.environ.get("GAT_TRACE")),
                               tmpdir=os.environ.get("GAT_TMPDIR"))
    if os.environ.get("GAT_TRACE"):
        print(f"HW exec time: {res.exec_time_ns} ns")

    out = np.empty((N, HEADS * OUT_F), np.float32)
    for c in range(NCORES):
        out[per_core[c]["perm_owned"]] = res.results[c]["out"][:NODES_PC]
    return out
